# revision 1
# baseline (speedup 1.0000x reference)
"""Trainium2 Bass kernel for CausalSelfAttentionModern (GQA + RoPE + causal SDPA).

Sharding: tensor-parallel over heads across 8 NeuronCores.
Device d owns q-heads {2d, 2d+1} and kv-head d//2.
Each device computes its heads' attention plus its slice of the output
projection (row-parallel); the host sums the 8 partial outputs.

All matmuls run as float32r (full-rate fp32 mode on the PE array).
"""

import numpy as np
import concourse.bacc as bacc
import concourse.tile as tile
import concourse.mybir as mybir
from concourse.bass_utils import run_bass_kernel_spmd

F32 = mybir.dt.float32
F32R = mybir.dt.float32r
EXP = mybir.ActivationFunctionType.Exp

# hardcoded problem shapes
T = 2048          # sequence length
C = 2048          # embedding dim
DH = 128          # head dim
NH = 16           # query heads
NKV = 4           # kv heads
N_CORES = 8
HPD = NH // N_CORES  # q-heads per device = 2
ROPE_BASE = 10000.0
SCALE = 1.0 / np.sqrt(DH)

NQ = 4            # t-quarters for projection phase
TQ = T // NQ      # 512
NW = 4            # attention tq windows
TW = T // NW      # 512
NCT = C // 128    # 16 contraction tiles
NTC = T // 128    # 16 token chunks


def _emit(nc):
    xT = nc.dram_tensor("xT", [C, T], F32R, kind="ExternalInput").ap()
    wq = nc.dram_tensor("wq", [128, NCT * HPD * DH], F32R, kind="ExternalInput").ap()
    wk = nc.dram_tensor("wk", [128, NCT * DH], F32R, kind="ExternalInput").ap()
    wv = nc.dram_tensor("wv", [128, NCT * DH], F32R, kind="ExternalInput").ap()
    wp = nc.dram_tensor("wp", [128, HPD * C], F32R, kind="ExternalInput").ap()
    cosT = nc.dram_tensor("cosT", [128, T], F32, kind="ExternalInput").ap()
    sinR = nc.dram_tensor("sinR", [128, T], F32, kind="ExternalInput").ap()
    ones = nc.dram_tensor("ones", [128, 128], F32R, kind="ExternalInput").ap()
    ident = nc.dram_tensor("ident", [128, 128], F32, kind="ExternalInput").ap()
    out = nc.dram_tensor("out", [T, C], F32, kind="ExternalOutput").ap()

    with tile.TileContext(nc) as tc:
        with (
            tc.tile_pool(name="cst", bufs=1) as cst,
            tc.tile_pool(name="ps", bufs=8, space="PSUM") as ps,
        ):
            # persistent SBUF tensors (DMAs emitted at first-use points below)
            cos_sb = cst.tile([128, T], F32, tag="cos")
            sin_sb = cst.tile([128, T], F32, tag="sin")
            ones_sb = cst.tile([128, 128], F32R, tag="ones")
            id_sb = cst.tile([128, 128], F32, tag="ident")
            wp_sb = cst.tile([128, HPD * C], F32R, tag="wp")

            qt_sb = [cst.tile([128, T], F32R, tag=f"qt{m}", name=f"qt{m}")
                     for m in range(HPD)]
            kt_sb = cst.tile([128, T], F32R, tag="kt")
            vtp_pool = None  # vt quarter tiles come from the rope pool
            v_sb = cst.tile([128, NTC * DH], F32R, tag="v")
            yt_sb = [cst.tile([128, T], F32R, tag=f"yt{m}", name=f"yt{m}")
                     for m in range(HPD)]

            # ---------------- projections + rope, per t-quarter ----------------
            with (
                tc.tile_pool(name="wqkv", bufs=1) as wqkv,
                tc.tile_pool(name="xts", bufs=5) as xts,
                tc.tile_pool(name="rope", bufs=1) as rope,
            ):
                wq_sb = wqkv.tile([128, NCT * HPD * DH], F32R, tag="wq")
                wk_sb = wqkv.tile([128, NCT * DH], F32R, tag="wk")
                wv_sb = wqkv.tile([128, NCT * DH], F32R, tag="wv")
                # weights on the ACT ring: c-tile groups so sems fire early
                for a, b in [(0, 2), (2, 4), (4, 7), (7, 11), (11, 16)]:
                    q1 = HPD * DH
                    nc.scalar.dma_start(wq_sb[:, a * q1:b * q1], wq[:, a * q1:b * q1])
                    nc.scalar.dma_start(wk_sb[:, a * DH:b * DH], wk[:, a * DH:b * DH])
                    nc.scalar.dma_start(wv_sb[:, a * DH:b * DH], wv[:, a * DH:b * DH])
                    if a == 7:
                        nc.scalar.dma_start(cos_sb[0:64, :], cosT[0:64, :])
                        nc.scalar.dma_start(sin_sb[0:64, :], sinR[0:64, :])

                xt_tiles = {}
                vt_tiles = {}

                def emit_xt_loads(qq):
                    # per half (8 c-tiles x 256 tokens) strided load
                    SW = TQ // 2
                    tsl = slice(qq * SW, (qq + 1) * SW)
                    for half in range(2):
                        xt = xts.tile([128, 8 * SW], F32R, tag="xt",
                                      name=f"xtq{qq}_{half}")
                        c0 = half * 8
                        splits = [(0, 3), (3, 8)] if (qq == 0 and half == 0) else [(0, 8)]
                        for a, b in splits:
                            nc.sync.dma_start(
                                xt[:, a * SW:b * SW].rearrange("p (ct t) -> p ct t", t=SW),
                                xT[(c0 + a) * 128:(c0 + b) * 128, tsl].rearrange(
                                    "(ct p) t -> p ct t", p=128))
                        xt_tiles[(qq, half)] = xt

                def emit_proj_subq(i, hooks=()):
                    # one 256-token sub-quarter: psum tiles complete before rope
                    SW = TQ // 2
                    tsl = slice(i * SW, (i + 1) * SW)
                    pq = [ps.tile([128, SW], F32, tag="ps", name=f"pq{i}_{m}")
                          for m in range(HPD)]
                    pk = ps.tile([128, SW], F32, tag="ps", name=f"pk{i}")
                    pv = ps.tile([128, SW], F32, tag="ps", name=f"pv{i}")
                    hooks = dict(hooks)
                    for ct in range(NCT):
                        fn = hooks.pop(ct, None)
                        if fn is not None:
                            fn()
                        xt = xt_tiles[(i, ct // 8)]
                        xsl = slice((ct % 8) * SW, (ct % 8 + 1) * SW)
                        st = ct == 0
                        sp = ct == NCT - 1
                        for m in range(HPD):
                            nc.tensor.matmul(
                                pq[m][:],
                                wq_sb[:, (ct * HPD + m) * DH:(ct * HPD + m + 1) * DH],
                                xt[:, xsl], start=st, stop=sp)
                        nc.tensor.matmul(
                            pk[:], wk_sb[:, ct * DH:(ct + 1) * DH],
                            xt[:, xsl], start=st, stop=sp)
                        nc.tensor.matmul(
                            pv[:], wv_sb[:, ct * DH:(ct + 1) * DH],
                            xt[:, xsl], start=st, stop=sp)

                    if i == 0:
                        # derive other halves (cos repeats, sin negates)
                        nc.scalar.copy(cos_sb[64:128, :], cos_sb[0:64, :])
                        nc.scalar.mul(sin_sb[64:128, :], sin_sb[0:64, :], -1.0)
                        nc.scalar.dma_start(id_sb[:], ident[:])
                        nc.scalar.dma_start(ones_sb[:], ones[:])
                    if i == 1:
                        nc.scalar.dma_start(wp_sb[:], wp[:])

                    # rope: out = psum*cos + shift(psum)*sinR  (shift = rotate-half)
                    for psrc, dst in [(pq[0], qt_sb[0]), (pq[1], qt_sb[1]), (pk, kt_sb)]:
                        cr = rope.tile([128, SW], F32, tag="crope")
                        nc.vector.tensor_mul(cr[:], psrc[:], cos_sb[:, tsl])
                        ur = rope.tile([128, SW], F32, tag="urot")
                        nc.vector.tensor_mul(ur[0:64, :], psrc[64:128, :], sin_sb[0:64, tsl])
                        nc.vector.tensor_mul(ur[64:128, :], psrc[0:64, :], sin_sb[64:128, tsl])
                        nc.vector.tensor_add(dst[:, tsl], cr[:], ur[:])
                    # v: plain copy to SBUF (fp32, feeds PE transpose)
                    vt_q = rope.tile([128, SW], F32, tag="vtq", name=f"vtq{i}")
                    nc.scalar.copy(vt_q[:], pv[:])
                    vt_tiles[i] = vt_q

                def emit_v_transpose(i):
                    # transpose V^T -> V for sub-quarter i (2 token chunks)
                    SW = TQ // 2
                    pvt = ps.tile([128, SW], F32, tag="ps", name=f"pvt{i}")
                    for j in range(2):
                        nc.tensor.transpose(
                            pvt[:, j * 128:(j + 1) * 128],
                            vt_tiles[i][:, j * 128:(j + 1) * 128],
                            id_sb[:])
                    nc.vector.tensor_copy(v_sb[:, i * SW:(i + 1) * SW], pvt[:])

                def emit_attn_head(tw0, twl, h):
                    wsl = slice(tw0, tw0 + twl)
                    nch = (tw0 + twl) // 128
                    w = tw0 // 128  # first diagonal chunk index
                    if True:
                        # phase 1: scores -> exp -> causal-zero, decoupled from pV
                        # chunk PAIRS share one psum bank and one exp instruction
                        pts = []
                        for cp in range(0, nch, 2):
                            npair = min(2, nch - cp)
                            pw = npair * twl
                            sc_ps = ps.tile([128, pw], F32, tag="ps",
                                            name=f"sc{w}_{h}_{cp}")
                            for k2 in range(npair):
                                cc = cp + k2
                                nc.tensor.matmul(
                                    sc_ps[:, k2 * twl:(k2 + 1) * twl],
                                    kt_sb[:, cc * 128:(cc + 1) * 128],
                                    qt_sb[h][:, wsl], start=True, stop=True)
                            pt = ptp.tile([128, pw], F32R, tag="pt",
                                          name=f"pt{w}_{h}_{cp}")
                            nc.scalar.activation(pt[:], sc_ps[:], EXP,
                                                 scale=float(SCALE))
                            for k2 in range(npair):
                                cc = cp + k2
                                if cc >= w:
                                    # zero strictly-above-diagonal: keep tq >= tk
                                    nc.gpsimd.affine_select(
                                        out=pt[:, k2 * twl:(k2 + 1) * twl],
                                        in_=pt[:, k2 * twl:(k2 + 1) * twl],
                                        pattern=[[1, twl]],
                                        compare_op=mybir.AluOpType.is_ge, fill=0.0,
                                        base=tw0 - cc * 128, channel_multiplier=-1)
                                pts.append((pt, k2 * twl))
                        # phase 2: y^T += V^T-chunks @ probs, sums via ones-matmul
                        y_ps = ps.tile([128, twl], F32, tag="ps", name=f"y{w}_{h}")
                        s_ps = ps.tile([128, twl], F32, tag="ps", name=f"s{w}_{h}")
                        for cc in range(nch):
                            st = cc == 0
                            sp = cc == nch - 1
                            pt, off = pts[cc]
                            psl = slice(off, off + twl)
                            nc.tensor.matmul(
                                y_ps[:], v_sb[:, cc * DH:(cc + 1) * DH],
                                pt[:, psl], start=st, stop=sp)
                            nc.tensor.matmul(
                                s_ps[:], ones_sb[:], pt[:, psl],
                                start=st, stop=sp)
                        rc = rcp.tile([128, twl], F32, tag="rc", name=f"rc{w}_{h}")
                        nc.vector.reciprocal(rc[:], s_ps[:])
                        nc.vector.tensor_mul(yt_sb[h][:, wsl], y_ps[:], rc[:])

                def emit_outproj_window(tw0, twl, split_out=False, rev=False,
                                        per_e=False):
                    jorder = range(twl // 128)
                    for j in (reversed(jorder) if rev else jorder):
                        t0 = tw0 + j * 128
                        ost = ostp.tile([128, C], F32, tag="ost", name=f"ost{t0}")
                        if per_e:
                            # hold one psum bank at a time (weavable inside proj)
                            for e in range(4):
                                po = ps.tile([128, 512], F32, tag="ps",
                                             name=f"po{t0}_{e}")
                                for k in range(HPD):
                                    nc.tensor.matmul(
                                        po[:],
                                        yt_sb[k][:, t0:t0 + 128],
                                        wp_sb[:, k * C + e * 512:k * C + (e + 1) * 512],
                                        start=(k == 0), stop=(k == HPD - 1))
                                nc.any.tensor_copy(ost[:, e * 512:(e + 1) * 512], po[:])
                        else:
                            po = [ps.tile([128, 512], F32, tag="ps",
                                          name=f"po{t0}_{e}") for e in range(4)]
                            for k in range(HPD):
                                for e in range(4):
                                    nc.tensor.matmul(
                                        po[e][:],
                                        yt_sb[k][:, t0:t0 + 128],
                                        wp_sb[:, k * C + e * 512:k * C + (e + 1) * 512],
                                        start=(k == 0), stop=(k == HPD - 1))
                            for e in range(4):
                                nc.any.tensor_copy(ost[:, e * 512:(e + 1) * 512], po[e][:])
                        if split_out:
                            for e in range(4):
                                esl = slice(e * 512, (e + 1) * 512)
                                nc.sync.dma_start(out[t0:t0 + 128, esl], ost[:, esl])
                        else:
                            nc.sync.dma_start(out[t0:t0 + 128, :], ost[:])

                with (
                    tc.tile_pool(name="pt", bufs=10) as ptp,
                    tc.tile_pool(name="rc", bufs=2) as rcp,
                    tc.tile_pool(name="ost", bufs=3) as ostp,
                ):
                    SW = TQ // 2
                    for qq in range(2 * NQ):
                        emit_xt_loads(qq)
                    for i in range(2 * NQ):
                        if i >= 1:
                            w0 = (i - 1) * SW

                            def mid_a(w0=w0, i=i):
                                emit_v_transpose(i - 1)
                                emit_attn_head(w0, SW, 0)

                            def mid_b(w0=w0):
                                emit_attn_head(w0, SW, 1)
                            c0 = 4 if i <= 3 else 2
                            hooks = ((c0, mid_a), (14 if i <= 3 else 13, mid_b))
                        else:
                            hooks = ()
                        emit_proj_subq(i, hooks=hooks)
                        if i >= 2:
                            emit_outproj_window((i - 2) * SW, SW, per_e=True)
                    emit_v_transpose(2 * NQ - 1)
                    emit_attn_head((2 * NQ - 1) * SW, SW, 0)
                    emit_outproj_window((2 * NQ - 2) * SW, SW, split_out=True, per_e=True)
                    emit_attn_head((2 * NQ - 1) * SW, SW, 1)
                    emit_outproj_window((2 * NQ - 1) * SW, SW, split_out=True, rev=True, per_e=True)

    nc.compile()
    return nc


_CACHE = {}


def _get_module():
    if "nc" not in _CACHE:
        nc = bacc.Bacc("TRN2", target_bir_lowering=False, debug=False)
        _CACHE["nc"] = _emit(nc)
    return _CACHE["nc"]


def _host_constants():
    if "consts" in _CACHE:
        return _CACHE["consts"]
    inv_freq = 1.0 / (ROPE_BASE ** (np.arange(0, DH, 2, dtype=np.float64) / DH))
    ang = np.outer(np.arange(T, dtype=np.float64), inv_freq)      # (T, 64)
    emb = np.concatenate([ang, ang], axis=-1)                     # (T, 128)
    cos = np.cos(emb).astype(np.float32)                          # (T, 128)
    sin = np.sin(emb).astype(np.float32)
    cosT = np.ascontiguousarray(cos.T)                            # (128, T)
    sinT = np.ascontiguousarray(sin.T)
    sign = np.where(np.arange(DH) < DH // 2, -1.0, 1.0).astype(np.float32)
    sinR = np.ascontiguousarray(sinT * sign[:, None])
    ones = np.ones((128, 128), dtype=np.float32)
    ident = np.eye(128, dtype=np.float32)
    _CACHE["consts"] = (cosT, sinR, ones, ident)
    return _CACHE["consts"]


def kernel(x, wq, wk, wv, wproj):
    x = np.asarray(x, dtype=np.float32)
    wq = np.asarray(wq, dtype=np.float32)
    wk = np.asarray(wk, dtype=np.float32)
    wv = np.asarray(wv, dtype=np.float32)
    wproj = np.asarray(wproj, dtype=np.float32)

    nc = _get_module()
    cosT, sinR, ones, ident = _host_constants()
    xT = np.ascontiguousarray(x[0].T)                             # (C, T)

    in_maps = []
    for d in range(N_CORES):
        h0 = HPD * d
        g = d // 2
        # wq columns for heads h0..h0+HPD-1 -> [128, NCT*HPD*DH] (c-tile major)
        wq_d = wq[:, h0 * DH:(h0 + HPD) * DH]                     # (C, HPD*DH)
        wq_l = np.ascontiguousarray(
            wq_d.reshape(NCT, 128, HPD * DH).transpose(1, 0, 2).reshape(128, -1))
        wk_d = wk[:, g * DH:(g + 1) * DH]
        wk_l = np.ascontiguousarray(
            wk_d.reshape(NCT, 128, DH).transpose(1, 0, 2).reshape(128, -1))
        wv_d = wv[:, g * DH:(g + 1) * DH]
        wv_l = np.ascontiguousarray(
            wv_d.reshape(NCT, 128, DH).transpose(1, 0, 2).reshape(128, -1))
        # wproj rows for our heads -> [128, HPD*C] (head-major free dim)
        wp_d = wproj[h0 * DH:(h0 + HPD) * DH, :]                  # (HPD*DH, C)
        wp_l = np.ascontiguousarray(
            wp_d.reshape(HPD, 128, C).transpose(1, 0, 2).reshape(128, -1))
        in_maps.append({
            "xT": xT, "wq": wq_l, "wk": wk_l, "wv": wv_l, "wp": wp_l,
            "cosT": cosT, "sinR": sinR, "ones": ones, "ident": ident,
        })

    res = run_bass_kernel_spmd(nc, in_maps, core_ids=list(range(N_CORES)))
    acc = res.results[0]["out"].astype(np.float32)
    for d in range(1, N_CORES):
        acc = acc + res.results[d]["out"].astype(np.float32)
    return acc.reshape(1, T, C)



# revision 3
# speedup vs baseline: 1.0583x; 1.0583x over previous
"""Trainium2 Bass kernel for CausalSelfAttentionModern (GQA + RoPE + causal SDPA).

Sharding: tensor-parallel over heads across 8 NeuronCores.
Device d owns q-heads {2d, 2d+1} and kv-head d//2.
Each device computes its heads' attention plus its slice of the output
projection (row-parallel); the host sums the 8 partial outputs.

v2: bf16 data path (DMA + matmul inputs; fp32 PSUM accumulation),
softmax denominator via tiny stationary-pt matmuls instead of a full
ones-matmul, V projected directly in [token, dh] layout, RoPE through
SBUF so DVE fast modes apply, causal mask via constant-mask multiply.
"""

import numpy as np
import ml_dtypes
import concourse.bacc as bacc
import concourse.tile as tile
import concourse.mybir as mybir
from concourse.bass_utils import run_bass_kernel_spmd

F32 = mybir.dt.float32
BF16 = mybir.dt.bfloat16
EXP = mybir.ActivationFunctionType.Exp
MULT = mybir.AluOpType.mult
ADD = mybir.AluOpType.add

# hardcoded problem shapes
T = 2048          # sequence length
C = 2048          # embedding dim
DH = 128          # head dim
NH = 16           # query heads
NKV = 4           # kv heads
N_CORES = 8
HPD = NH // N_CORES  # q-heads per device = 2
ROPE_BASE = 10000.0
SCALE = 1.0 / np.sqrt(DH)

NCT = C // 128    # 16 contraction tiles
NTC = T // 128    # 16 token chunks
SW = 256          # sub-quarter / attention window width
NSQ = T // SW     # 8 windows

BF = ml_dtypes.bfloat16


def _emit(nc):
    xT = nc.dram_tensor("xT", [C, T], BF16, kind="ExternalInput").ap()
    wq = nc.dram_tensor("wq", [128, NCT * HPD * DH], BF16, kind="ExternalInput").ap()
    wk = nc.dram_tensor("wk", [128, NCT * DH], BF16, kind="ExternalInput").ap()
    wv = nc.dram_tensor("wv", [128, NCT * DH], BF16, kind="ExternalInput").ap()
    wp = nc.dram_tensor("wp", [128, HPD * C], BF16, kind="ExternalInput").ap()
    cosT = nc.dram_tensor("cosT", [128, T], BF16, kind="ExternalInput").ap()
    sinR = nc.dram_tensor("sinR", [128, T], BF16, kind="ExternalInput").ap()
    ones = nc.dram_tensor("ones", [128, 128], BF16, kind="ExternalInput").ap()
    identf = nc.dram_tensor("identf", [128, 128], F32, kind="ExternalInput").ap()
    masks = nc.dram_tensor("masks", [128, 2 * SW], BF16, kind="ExternalInput").ap()
    out = nc.dram_tensor("out", [T, C], BF16, kind="ExternalOutput").ap()

    with tile.TileContext(nc) as tc:
        with (
            tc.tile_pool(name="cst", bufs=1) as cst,
            tc.tile_pool(name="ps", bufs=1, space="PSUM") as ps,
            tc.tile_pool(name="xts", bufs=5) as xts,
            tc.tile_pool(name="rope", bufs=4) as rope,
            tc.tile_pool(name="pts", bufs=12) as pts,
            tc.tile_pool(name="osts", bufs=3) as osts,
            tc.tile_pool(name="rcs", bufs=2) as rcs,
        ):
            # persistent SBUF tensors
            cos_sb = cst.tile([128, T], BF16, tag="cos")
            sin_sb = cst.tile([128, T], BF16, tag="sin")
            ones_sb = cst.tile([128, 128], BF16, tag="ones")
            identf_sb = cst.tile([128, 128], F32, tag="identf")
            masks_sb = cst.tile([128, 2 * SW], BF16, tag="masks")
            scratch = cst.tile([128, 128], BF16, tag="scratch")
            wq_sb = cst.tile([128, NCT * HPD * DH], BF16, tag="wq")
            wk_sb = cst.tile([128, NCT * DH], BF16, tag="wk")
            wv_sb = cst.tile([128, NCT * DH], BF16, tag="wv")
            wp_sb = cst.tile([128, HPD * C], BF16, tag="wp")

            qt_sb = [cst.tile([128, T], BF16, tag=f"qt{m}", name=f"qt{m}")
                     for m in range(HPD)]
            kt_sb = cst.tile([128, T], BF16, tag="kt")
            v_sb = cst.tile([128, NTC * DH], BF16, tag="v")
            yt_sb = [cst.tile([128, T], BF16, tag=f"yt{m}", name=f"yt{m}")
                     for m in range(HPD)]

            # ---- PE warmup: ramp the p-state before real work arrives ----
            nc.vector.memset(scratch[:], 0)
            warm_ps = ps.tile([128, 512], F32, tag="sc", bufs=2, name="warm")
            for wdx in range(12):
                nc.tensor.matmul(warm_ps[:, 0:128], scratch[:], scratch[:],
                                 start=True, stop=True, skip_group_check=True)

            # ---- DMA: weights early, x tiles streamed ----
            def emit_weight_loads():
                q1 = HPD * DH
                for a, b in [(0, 2), (2, 4), (4, 8), (8, 12), (12, 16)]:
                    nc.sync.dma_start(wq_sb[:, a * q1:b * q1], wq[:, a * q1:b * q1])
                    nc.sync.dma_start(wk_sb[:, a * DH:b * DH], wk[:, a * DH:b * DH])
                    nc.sync.dma_start(wv_sb[:, a * DH:b * DH], wv[:, a * DH:b * DH])
                    if a == 4:
                        nc.scalar.dma_start(cos_sb[:], cosT[:])
                        nc.scalar.dma_start(sin_sb[:], sinR[:])
                    if a == 8:
                        nc.scalar.dma_start(ones_sb[:], ones[:])
                        nc.scalar.dma_start(identf_sb[:], identf[:])
                        nc.scalar.dma_start(masks_sb[:], masks[:])
                    if a == 12:
                        nc.scalar.dma_start(wp_sb[:], wp[:])

            xt_tiles = {}

            def emit_xt_load(i):
                # one sub-quarter = 2 half tiles of 8 c-tiles x 256 tokens
                tsl = slice(i * SW, (i + 1) * SW)
                for half in range(2):
                    xt = xts.tile([128, 8 * SW], BF16, tag="xt",
                                  name=f"xt{i}_{half}")
                    c0 = half * 8
                    splits = [(0, 3), (3, 8)] if (i == 0 and half == 0) else [(0, 8)]
                    for a, b in splits:
                        nc.sync.dma_start(
                            xt[:, a * SW:b * SW].rearrange("p (ct t) -> p ct t", t=SW),
                            xT[(c0 + a) * 128:(c0 + b) * 128, tsl].rearrange(
                                "(ct p) t -> p ct t", p=128))
                    xt_tiles[(i, half)] = xt

            def emit_proj(i):
                # q0,q1 -> pqq bank; k, v(2 token chunks) -> pkv bank
                tsl = slice(i * SW, (i + 1) * SW)
                pqq = ps.tile([128, 512], F32, tag="pqq", bufs=1, name=f"pqq{i}")
                pkv = ps.tile([128, 512], F32, tag="pkv", bufs=1, name=f"pkv{i}")
                for ct in range(NCT):
                    xt = xt_tiles[(i, ct // 8)]
                    xsl = slice((ct % 8) * SW, (ct % 8 + 1) * SW)
                    st = ct == 0
                    sp = ct == NCT - 1
                    for m in range(HPD):
                        nc.tensor.matmul(
                            pqq[:, m * SW:(m + 1) * SW],
                            wq_sb[:, (ct * HPD + m) * DH:(ct * HPD + m + 1) * DH],
                            xt[:, xsl], start=st, stop=sp, skip_group_check=True)
                    nc.tensor.matmul(
                        pkv[:, 0:SW], wk_sb[:, ct * DH:(ct + 1) * DH],
                        xt[:, xsl], start=st, stop=sp, skip_group_check=True)
                    # v in [token, dh] layout: x chunk stationary, wv moving
                    for j in range(2):
                        nc.tensor.matmul(
                            pkv[:, SW + j * DH:SW + (j + 1) * DH],
                            xt[:, (ct % 8) * SW + j * 128:(ct % 8) * SW + (j + 1) * 128],
                            wv_sb[:, ct * DH:(ct + 1) * DH],
                            start=st, stop=sp, skip_group_check=True)

                # rope: copy psum->sbuf bf16, then 4x-mode DVE ops
                for idx, dst in [(0, qt_sb[0]), (1, qt_sb[1]), (2, kt_sb)]:
                    src = pkv if dst is kt_sb else pqq
                    psl = slice(0, SW) if idx != 1 else slice(SW, 2 * SW)
                    p_sb = rope.tile([128, SW], BF16, tag="p_sb")
                    nc.any.tensor_copy(p_sb[:], src[:, psl])
                    cr = rope.tile([128, SW], BF16, tag="cr")
                    nc.vector.scalar_tensor_tensor(
                        cr[:], p_sb[:], 1.0, cos_sb[:, tsl], op0=MULT, op1=MULT)
                    ur = rope.tile([128, SW], BF16, tag="ur")
                    nc.vector.scalar_tensor_tensor(
                        ur[0:64, :], p_sb[64:128, :], 1.0, sin_sb[0:64, tsl],
                        op0=MULT, op1=MULT)
                    nc.vector.scalar_tensor_tensor(
                        ur[64:128, :], p_sb[0:64, :], 1.0, sin_sb[64:128, tsl],
                        op0=MULT, op1=MULT)
                    nc.vector.scalar_tensor_tensor(
                        dst[:, tsl], cr[:], 1.0, ur[:], op0=MULT, op1=ADD)
                # v: plain copies to [token, dh] sbuf
                for j in range(2):
                    tc0 = 2 * i + j
                    nc.any.tensor_copy(v_sb[:, tc0 * DH:(tc0 + 1) * DH],
                                       pkv[:, SW + j * DH:SW + (j + 1) * DH])

            def weave(po_units):
                if po_units:
                    po_units.pop(0)()

            # ---------------- attention ----------------
            def emit_attn_scores(w, h, po_units):
                """Scores + exp (+mask) for window w, head h; weaves po_units."""
                tw0 = w * SW
                wsl = slice(tw0, tw0 + SW)
                npair = w + 1
                pt_list = []
                for cp in range(npair):
                    sc_ps = ps.tile([128, 512], F32, tag="sc", bufs=2,
                                    name=f"sc{w}_{h}_{cp}")
                    for k2 in range(2):
                        cc = 2 * cp + k2
                        nc.tensor.matmul(
                            sc_ps[:, k2 * SW:(k2 + 1) * SW],
                            kt_sb[:, cc * 128:(cc + 1) * 128],
                            qt_sb[h][:, wsl], start=True, stop=True)
                    pt = pts.tile([128, 512], BF16, tag="pt", name=f"pt{w}_{h}_{cp}")
                    nc.scalar.activation(pt[:], sc_ps[:], EXP, scale=float(SCALE))
                    if cp == npair - 1:
                        # diagonal pair: zero strictly-above-diagonal
                        nc.vector.scalar_tensor_tensor(
                            pt[:], pt[:], 1.0, masks_sb[:], op0=MULT, op1=MULT)
                    pt_list.append(pt)
                    if cp % 2 == 1:
                        weave(po_units)
                return pt_list

            def emit_attn_tail(w, h, pt_list, po_units):
                """y/s accumulation + softmax normalization for window w, head h."""
                tw0 = w * SW
                wsl = slice(tw0, tw0 + SW)
                nch = 2 * (w + 1)
                ys = ps.tile([128, 512], F32, tag="ys", bufs=2, name=f"ys{w}_{h}")
                for cc in range(nch):
                    st = cc == 0
                    sp = cc == nch - 1
                    pt = pt_list[cc // 2]
                    psl = slice((cc % 2) * SW, (cc % 2 + 1) * SW)
                    nc.tensor.matmul(
                        ys[:, 0:SW], v_sb[:, cc * DH:(cc + 1) * DH],
                        pt[:, psl], start=st, stop=sp, skip_group_check=True)
                    # denominator partial sums: stationary-pt, 1-row moving ones
                    for h2 in range(2):
                        nc.tensor.matmul(
                            ys[:, SW + h2:SW + h2 + 1],
                            pt[:, (cc % 2) * SW + h2 * 128:(cc % 2) * SW + (h2 + 1) * 128],
                            ones_sb[:, 0:1], start=st, stop=sp,
                            skip_group_check=True)
                # rc chain: s -> sbuf -> transpose -> recip -> broadcast
                s_sb = rcs.tile([128, 2], F32, tag="s_sb", name=f"ssb{w}_{h}")
                nc.vector.tensor_copy(s_sb[:], ys[:, SW:SW + 2])
                weave(po_units)
                rc_ps = ps.tile([128, 512], F32, tag="sc", bufs=2,
                                name=f"rcps{w}_{h}")
                for h2 in range(2):
                    nc.tensor.transpose(rc_ps[0:1, h2 * 128:(h2 + 1) * 128],
                                        s_sb[:, h2:h2 + 1], identf_sb[:])
                weave(po_units)
                rcT_sb = rcs.tile([1, 256], BF16, tag="rcT", name=f"rcT{w}_{h}")
                with nc.allow_low_precision("softmax denominators; bf16 scale is plenty"):
                    nc.vector.reciprocal(rcT_sb[:], rc_ps[0:1, 0:256])
                for h2 in range(2):
                    nc.tensor.matmul(
                        rc_ps[:, 256 + h2 * 128:256 + (h2 + 1) * 128],
                        ones_sb[0:1, :], rcT_sb[:, h2 * 128:(h2 + 1) * 128],
                        start=True, stop=True, skip_group_check=True)
                nc.vector.tensor_mul(yt_sb[h][:, wsl], ys[:, 0:SW],
                                     rc_ps[:, 256:512])

            # ---------------- output projection ----------------
            def make_po_units(w):
                """Per 128-token chunk x 512-col block outproj units."""
                units = []
                for j in range(2):
                    t0 = w * SW + j * 128
                    ost = osts.tile([128, C], BF16, tag="ost", name=f"ost{t0}")
                    for e in range(4):
                        def unit(t0=t0, ost=ost, e=e):
                            po = ps.tile([128, 512], F32, tag="po", bufs=2,
                                         name=f"po{t0}_{e}")
                            for k in range(HPD):
                                nc.tensor.matmul(
                                    po[:],
                                    yt_sb[k][:, t0:t0 + 128],
                                    wp_sb[:, k * C + e * 512:k * C + (e + 1) * 512],
                                    start=(k == 0), stop=(k == HPD - 1))
                            nc.any.tensor_copy(ost[:, e * 512:(e + 1) * 512], po[:])
                        units.append(unit)

                    def dma_unit(t0=t0, ost=ost):
                        nc.scalar.dma_start(out[t0:t0 + 128, :], ost[:])
                    units.append(dma_unit)
                return units

            def emit_attn_window(w, po_units):
                pt0 = emit_attn_scores(w, 0, po_units)
                emit_attn_tail(w, 0, pt0, po_units)
                pt1 = emit_attn_scores(w, 1, po_units)
                emit_attn_tail(w, 1, pt1, po_units)
                for u in po_units:
                    u()

            # ---------------- main loop ----------------
            emit_weight_loads()
            for i in range(3):
                emit_xt_load(i)
            for i in range(NSQ):
                if i + 3 < NSQ:
                    emit_xt_load(i + 3)
                emit_proj(i)
                if i >= 1:
                    emit_attn_window(i - 1, make_po_units(i - 2) if i >= 2 else [])
            # tail: window 7 + outproj windows 6, 7
            emit_attn_window(NSQ - 1, make_po_units(NSQ - 2))
            for u in make_po_units(NSQ - 1):
                u()

    nc.compile()
    return nc


_CACHE = {}


def _get_module():
    if "nc" not in _CACHE:
        nc = bacc.Bacc("TRN2", target_bir_lowering=False, debug=False)
        _CACHE["nc"] = _emit(nc)
    return _CACHE["nc"]


def _host_constants():
    if "consts" in _CACHE:
        return _CACHE["consts"]
    inv_freq = 1.0 / (ROPE_BASE ** (np.arange(0, DH, 2, dtype=np.float64) / DH))
    ang = np.outer(np.arange(T, dtype=np.float64), inv_freq)      # (T, 64)
    emb = np.concatenate([ang, ang], axis=-1)                     # (T, 128)
    cos = np.cos(emb).astype(np.float32)                          # (T, 128)
    sin = np.sin(emb).astype(np.float32)
    cosT = np.ascontiguousarray(cos.T).astype(BF)                 # (128, T)
    sign = np.where(np.arange(DH) < DH // 2, -1.0, 1.0).astype(np.float32)
    sinR = np.ascontiguousarray(sin.T * sign[:, None]).astype(BF)
    ones = np.ones((128, 128), dtype=BF)
    identf = np.eye(128, dtype=np.float32)
    # causal masks for the diagonal chunk pair of each 256-wide window
    j = np.arange(SW)[None, :]
    p = np.arange(128)[:, None]
    m0 = (j >= p).astype(BF)
    m1 = (j >= p + 128).astype(BF)
    masks = np.concatenate([m0, m1], axis=1)                      # (128, 512)
    _CACHE["consts"] = (cosT, sinR, ones, identf, masks)
    return _CACHE["consts"]


def kernel(x, wq, wk, wv, wproj):
    x = np.asarray(x, dtype=np.float32)
    wq = np.asarray(wq, dtype=np.float32)
    wk = np.asarray(wk, dtype=np.float32)
    wv = np.asarray(wv, dtype=np.float32)
    wproj = np.asarray(wproj, dtype=np.float32)

    nc = _get_module()
    cosT, sinR, ones, identf, masks = _host_constants()
    xT = np.ascontiguousarray(x[0].T).astype(BF)                  # (C, T)

    in_maps = []
    for d in range(N_CORES):
        h0 = HPD * d
        g = d // 2
        # wq columns for heads h0..h0+HPD-1 -> [128, NCT*HPD*DH] (c-tile major)
        wq_d = wq[:, h0 * DH:(h0 + HPD) * DH]                     # (C, HPD*DH)
        wq_l = np.ascontiguousarray(
            wq_d.reshape(NCT, 128, HPD * DH).transpose(1, 0, 2).reshape(128, -1)
        ).astype(BF)
        wk_d = wk[:, g * DH:(g + 1) * DH]
        wk_l = np.ascontiguousarray(
            wk_d.reshape(NCT, 128, DH).transpose(1, 0, 2).reshape(128, -1)
        ).astype(BF)
        wv_d = wv[:, g * DH:(g + 1) * DH]
        wv_l = np.ascontiguousarray(
            wv_d.reshape(NCT, 128, DH).transpose(1, 0, 2).reshape(128, -1)
        ).astype(BF)
        # wproj rows for our heads -> [128, HPD*C] (head-major free dim)
        wp_d = wproj[h0 * DH:(h0 + HPD) * DH, :]                  # (HPD*DH, C)
        wp_l = np.ascontiguousarray(
            wp_d.reshape(HPD, 128, C).transpose(1, 0, 2).reshape(128, -1)
        ).astype(BF)
        in_maps.append({
            "xT": xT, "wq": wq_l, "wk": wk_l, "wv": wv_l, "wp": wp_l,
            "cosT": cosT, "sinR": sinR, "ones": ones, "identf": identf,
            "masks": masks,
        })

    res = run_bass_kernel_spmd(nc, in_maps, core_ids=list(range(N_CORES)))
    acc = res.results[0]["out"].astype(np.float32)
    for d in range(1, N_CORES):
        acc = acc + res.results[d]["out"].astype(np.float32)
    return acc.reshape(1, T, C)


# revision 7
# speedup vs baseline: 1.0808x; 1.0212x over previous
"""Trainium2 Bass kernel for CausalSelfAttentionModern (GQA + RoPE + causal SDPA).

Sharding: tensor-parallel over heads across 8 NeuronCores.
Device d owns q-heads {2d, 2d+1} and kv-head d//2.
Each device computes its heads' attention plus its slice of the output
projection (row-parallel); the host sums the 8 partial outputs.

v2: bf16 data path (DMA + matmul inputs; fp32 PSUM accumulation),
softmax denominator via tiny stationary-pt matmuls instead of a full
ones-matmul, V projected directly in [token, dh] layout, RoPE through
SBUF so DVE fast modes apply, causal mask via constant-mask multiply,
and a unified PE weave: projection of sub-quarter i and output
projection of window i-2 fill the latency bubbles of attention window
i-1 (exp pipeline, softmax-denominator chain).
"""

import numpy as np
import ml_dtypes
import concourse.bacc as bacc
import concourse.tile as tile
import concourse.mybir as mybir
from concourse.bass_utils import run_bass_kernel_spmd

F32 = mybir.dt.float32
BF16 = mybir.dt.bfloat16
EXP = mybir.ActivationFunctionType.Exp
MULT = mybir.AluOpType.mult
ADD = mybir.AluOpType.add

# hardcoded problem shapes
T = 2048          # sequence length
C = 2048          # embedding dim
DH = 128          # head dim
NH = 16           # query heads
NKV = 4           # kv heads
N_CORES = 8
HPD = NH // N_CORES  # q-heads per device = 2
ROPE_BASE = 10000.0
SCALE = 1.0 / np.sqrt(DH)

NCT = C // 128    # 16 contraction tiles
NTC = T // 128    # 16 token chunks
SW = 256          # sub-quarter / attention window width
NSQ = T // SW     # 8 windows

BF = ml_dtypes.bfloat16


def _emit(nc):
    xT = nc.dram_tensor("xT", [C, T], BF16, kind="ExternalInput").ap()
    wq = nc.dram_tensor("wq", [128, NCT * HPD * DH], BF16, kind="ExternalInput").ap()
    wk = nc.dram_tensor("wk", [128, NCT * DH], BF16, kind="ExternalInput").ap()
    wv = nc.dram_tensor("wv", [128, NCT * DH], BF16, kind="ExternalInput").ap()
    wp = nc.dram_tensor("wp", [128, HPD * C], BF16, kind="ExternalInput").ap()
    cosT = nc.dram_tensor("cosT", [128, T], BF16, kind="ExternalInput").ap()
    sinR = nc.dram_tensor("sinR", [128, T], BF16, kind="ExternalInput").ap()
    ones = nc.dram_tensor("ones", [128, 128], BF16, kind="ExternalInput").ap()
    identf = nc.dram_tensor("identf", [128, 128], F32, kind="ExternalInput").ap()
    masks = nc.dram_tensor("masks", [128, 2 * SW], BF16, kind="ExternalInput").ap()
    out = nc.dram_tensor("out", [T, C], BF16, kind="ExternalOutput").ap()

    with tile.TileContext(nc) as tc:
        with (
            tc.tile_pool(name="cst", bufs=1) as cst,
            tc.tile_pool(name="ps", bufs=1, space="PSUM") as ps,
            tc.tile_pool(name="xts", bufs=6) as xts,
            tc.tile_pool(name="rope", bufs=4) as rope,
            tc.tile_pool(name="pts", bufs=12) as pts,
            tc.tile_pool(name="osts", bufs=3) as osts,
            tc.tile_pool(name="rcs", bufs=2) as rcs,
        ):
            # persistent SBUF tensors
            cos_sb = cst.tile([128, T], BF16, tag="cos")
            sin_sb = cst.tile([128, T], BF16, tag="sin")
            ones_sb = cst.tile([128, 128], BF16, tag="ones")
            identf_sb = cst.tile([128, 128], F32, tag="identf")
            masks_sb = cst.tile([128, 2 * SW], BF16, tag="masks")
            scratch = cst.tile([128, 256], BF16, tag="scratch")
            wq_sb = cst.tile([128, NCT * HPD * DH], BF16, tag="wq")
            wk_sb = cst.tile([128, NCT * DH], BF16, tag="wk")
            wv_sb = cst.tile([128, NCT * DH], BF16, tag="wv")
            wp_sb = cst.tile([128, HPD * C], BF16, tag="wp")

            qt_sb = [cst.tile([128, T], BF16, tag=f"qt{m}", name=f"qt{m}")
                     for m in range(HPD)]
            kt_sb = cst.tile([128, T], BF16, tag="kt")
            v_sb = cst.tile([128, NTC * DH], BF16, tag="v")
            yt_sb = [cst.tile([128, T], BF16, tag=f"yt{m}", name=f"yt{m}")
                     for m in range(HPD)]

            # ---- PE warmup: ramp the p-state before real work arrives ----
            nc.vector.memset(scratch[:], 0)
            warm_ps = ps.tile([128, 512], F32, tag="sc", bufs=2, name="warm")
            for wdx in range(12):
                nc.tensor.matmul(warm_ps[:, 0:256], scratch[:, 0:128], scratch[:],
                                 start=True, stop=True, skip_group_check=True)

            # ---- DMA emission helpers ----
            xt_tiles = {}

            def emit_xt_load(i, first_splits=False):
                # one sub-quarter = 2 half tiles of 8 c-tiles x 256 tokens
                tsl = slice(i * SW, (i + 1) * SW)
                for half in range(2):
                    xt = xts.tile([128, 8 * SW], BF16, tag="xt",
                                  name=f"xt{i}_{half}")
                    xt_tiles[(i, half)] = xt

            def emit_xt_dma(i, half, a, b):
                tsl = slice(i * SW, (i + 1) * SW)
                xt = xt_tiles[(i, half)]
                c0 = half * 8
                nc.sync.dma_start(
                    xt[:, a * SW:b * SW].rearrange("p (ct t) -> p ct t", t=SW),
                    xT[(c0 + a) * 128:(c0 + b) * 128, tsl].rearrange(
                        "(ct p) t -> p ct t", p=128))

            def load_xt(i):
                emit_xt_load(i)
                emit_xt_dma(i, 0, 0, 8)
                emit_xt_dma(i, 1, 0, 8)

            def emit_startup_dmas():
                q1 = HPD * DH
                # first x slice + first weight c-tiles: unblock proj(0) asap
                emit_xt_load(0)
                emit_xt_dma(0, 0, 0, 3)
                nc.sync.dma_start(wq_sb[:, 0:2 * q1], wq[:, 0:2 * q1])
                nc.sync.dma_start(wk_sb[:, 0:2 * DH], wk[:, 0:2 * DH])
                nc.sync.dma_start(wv_sb[:, 0:2 * DH], wv[:, 0:2 * DH])
                emit_xt_dma(0, 0, 3, 8)
                nc.sync.dma_start(wq_sb[:, 2 * q1:9 * q1], wq[:, 2 * q1:9 * q1])
                emit_xt_dma(0, 1, 0, 8)
                nc.sync.dma_start(wk_sb[:, 2 * DH:16 * DH], wk[:, 2 * DH:16 * DH])
                nc.sync.dma_start(wv_sb[:, 2 * DH:16 * DH], wv[:, 2 * DH:16 * DH])
                nc.sync.dma_start(wq_sb[:, 9 * q1:16 * q1], wq[:, 9 * q1:16 * q1])
                # constants on the ACT ring in parallel
                nc.scalar.dma_start(cos_sb[:], cosT[:])
                nc.scalar.dma_start(sin_sb[:], sinR[:])
                nc.scalar.dma_start(ones_sb[:], ones[:])
                nc.scalar.dma_start(identf_sb[:], identf[:])
                nc.scalar.dma_start(masks_sb[:], masks[:])
                nc.scalar.dma_start(wp_sb[:], wp[:])
                load_xt(1)
                load_xt(2)

            # ---- projection units (PE filler) ----
            proj_psum = {}

            def proj_ct_unit(i, ct):
                def unit():
                    if ct == 0:
                        proj_psum[i] = (
                            ps.tile([128, 512], F32, tag="pqq", bufs=1,
                                    name=f"pqq{i}"),
                            ps.tile([128, 512], F32, tag="pkv", bufs=1,
                                    name=f"pkv{i}"),
                        )
                    pqq, pkv = proj_psum[i]
                    xt = xt_tiles[(i, ct // 8)]
                    xsl = slice((ct % 8) * SW, (ct % 8 + 1) * SW)
                    st = ct == 0
                    sp = ct == NCT - 1
                    for m in range(HPD):
                        nc.tensor.matmul(
                            pqq[:, m * SW:(m + 1) * SW],
                            wq_sb[:, (ct * HPD + m) * DH:(ct * HPD + m + 1) * DH],
                            xt[:, xsl], start=st, stop=sp, skip_group_check=True)
                    nc.tensor.matmul(
                        pkv[:, 0:SW], wk_sb[:, ct * DH:(ct + 1) * DH],
                        xt[:, xsl], start=st, stop=sp, skip_group_check=True)
                    # v in [token, dh] layout: x chunk stationary, wv moving
                    for j in range(2):
                        nc.tensor.matmul(
                            pkv[:, SW + j * DH:SW + (j + 1) * DH],
                            xt[:, (ct % 8) * SW + j * 128:(ct % 8) * SW + (j + 1) * 128],
                            wv_sb[:, ct * DH:(ct + 1) * DH],
                            start=st, stop=sp, skip_group_check=True)
                    if sp:
                        emit_rope_v(i)
                return unit

            def emit_rope_v(i):
                # rope: copy psum->sbuf bf16 (Pool), then 4x-mode DVE ops
                tsl = slice(i * SW, (i + 1) * SW)
                pqq, pkv = proj_psum[i]
                for idx, dst in [(0, qt_sb[0]), (1, qt_sb[1]), (2, kt_sb)]:
                    src = pkv if dst is kt_sb else pqq
                    psl = slice(0, SW) if idx != 1 else slice(SW, 2 * SW)
                    p_sb = rope.tile([128, SW], BF16, tag="p_sb")
                    nc.gpsimd.tensor_copy(p_sb[:], src[:, psl])
                    cr = rope.tile([128, SW], BF16, tag="cr")
                    nc.vector.scalar_tensor_tensor(
                        cr[:], p_sb[:], ones_sb[:, 0:1], cos_sb[:, tsl],
                        op0=MULT, op1=MULT)
                    ur = rope.tile([128, SW], BF16, tag="ur")
                    nc.vector.scalar_tensor_tensor(
                        ur[0:64, :], p_sb[64:128, :], ones_sb[64:128, 0:1],
                        sin_sb[0:64, tsl], op0=MULT, op1=MULT)
                    nc.vector.scalar_tensor_tensor(
                        ur[64:128, :], p_sb[0:64, :], ones_sb[0:64, 0:1],
                        sin_sb[64:128, tsl], op0=MULT, op1=MULT)
                    nc.vector.scalar_tensor_tensor(
                        dst[:, tsl], cr[:], ones_sb[:, 0:1], ur[:],
                        op0=MULT, op1=ADD)
                # v: plain copies to [token, dh] sbuf
                for j in range(2):
                    tc0 = 2 * i + j
                    nc.gpsimd.tensor_copy(v_sb[:, tc0 * DH:(tc0 + 1) * DH],
                                          pkv[:, SW + j * DH:SW + (j + 1) * DH])

            # ---- output projection units (PE filler) ----
            OST_ENG = [nc.scalar.copy, nc.vector.tensor_copy,
                       nc.gpsimd.tensor_copy, nc.vector.tensor_copy]

            def make_po_units(w):
                units = []
                for j in range(2):
                    t0 = w * SW + j * 128
                    ost = osts.tile([128, C], BF16, tag="ost", name=f"ost{t0}")
                    for e in range(4):
                        def unit(t0=t0, ost=ost, e=e):
                            po = ps.tile([128, 512], F32, tag="po", bufs=2,
                                         name=f"po{t0}_{e}")
                            for k in range(HPD):
                                nc.tensor.matmul(
                                    po[:],
                                    yt_sb[k][:, t0:t0 + 128],
                                    wp_sb[:, k * C + e * 512:k * C + (e + 1) * 512],
                                    start=(k == 0), stop=(k == HPD - 1))
                            OST_ENG[e](ost[:, e * 512:(e + 1) * 512], po[:])
                        units.append(unit)

                    def dma_unit(t0=t0, ost=ost):
                        nc.scalar.dma_start(out[t0:t0 + 128, :], ost[:])
                    units.append(dma_unit)
                return units

            # ---- attention units ----
            def make_attn_units(w):
                """Window w, both heads: scores/exp/mask, y/s, rc chain."""
                units = []
                npair = w + 1
                tw0 = w * SW
                wsl = slice(tw0, tw0 + SW)

                for h in range(HPD):
                    state = {"pts": {}, "ys": None}

                    def y_pair(h, cp, state):
                        nch = 2 * npair
                        ys = state["ys"]
                        pt = state["pts"][cp]
                        for k2 in range(2):
                            cc = 2 * cp + k2
                            st = cc == 0
                            sp = cc == nch - 1
                            psl = slice(k2 * SW, (k2 + 1) * SW)
                            nc.tensor.matmul(
                                ys[:, 0:SW], v_sb[:, cc * DH:(cc + 1) * DH],
                                pt[:, psl], start=st, stop=sp,
                                skip_group_check=True)
                            for h2 in range(2):
                                nc.tensor.matmul(
                                    ys[:, SW + h2:SW + h2 + 1],
                                    pt[:, k2 * SW + h2 * 128:k2 * SW + (h2 + 1) * 128],
                                    ones_sb[:, 0:1], start=st, stop=sp,
                                    skip_group_check=True)

                    def sc_unit(h=h, cp=0, state=state):
                        if cp == 0:
                            state["ys"] = ps.tile([128, 512], F32, tag="ys",
                                                  bufs=2, name=f"ys{w}_{h}")
                        sc_ps = ps.tile([128, 512], F32, tag="sc", bufs=2,
                                        name=f"sc{w}_{h}_{cp}")
                        for k2 in range(2):
                            cc = 2 * cp + k2
                            nc.tensor.matmul(
                                sc_ps[:, k2 * SW:(k2 + 1) * SW],
                                kt_sb[:, cc * 128:(cc + 1) * 128],
                                qt_sb[h][:, wsl], start=True, stop=True)
                        pt = pts.tile([128, 512], BF16, tag="pt",
                                      name=f"pt{w}_{h}_{cp}")
                        nc.scalar.activation(pt[:], sc_ps[:], EXP,
                                             scale=float(SCALE))
                        if cp == npair - 1:
                            # diagonal pair: zero strictly-above-diagonal
                            nc.vector.scalar_tensor_tensor(
                                pt[:], pt[:], ones_sb[:, 0:1], masks_sb[:],
                                op0=MULT, op1=MULT)
                        state["pts"][cp] = pt
                        if cp >= 1:
                            y_pair(h, cp - 1, state)

                    for cp in range(npair):
                        units.append(lambda h=h, cp=cp, state=state:
                                     sc_unit(h, cp, state))

                    def y_last(h=h, state=state):
                        y_pair(h, npair - 1, state)
                        s_sb = rcs.tile([128, 2], F32, tag="s_sb",
                                        name=f"ssb{w}_{h}")
                        nc.vector.tensor_copy(s_sb[:], state["ys"][:, SW:SW + 2])
                        state["s_sb"] = s_sb
                    units.append(y_last)

                    def rc1(h=h, state=state):
                        rc_ps = ps.tile([128, 512], F32, tag="sc", bufs=2,
                                        name=f"rcps{w}_{h}")
                        for h2 in range(2):
                            nc.tensor.transpose(
                                rc_ps[0:1, h2 * 128:(h2 + 1) * 128],
                                state["s_sb"][:, h2:h2 + 1], identf_sb[:])
                        state["rc_ps"] = rc_ps
                        rcT_sb = rcs.tile([1, 256], BF16, tag="rcT",
                                          name=f"rcT{w}_{h}")
                        with nc.allow_low_precision("softmax denominators"):
                            nc.vector.reciprocal(rcT_sb[:], rc_ps[0:1, 0:256])
                        state["rcT_sb"] = rcT_sb
                    units.append(rc1)

                    def rc2(h=h, state=state):
                        rc_ps = state["rc_ps"]
                        for h2 in range(2):
                            nc.tensor.matmul(
                                rc_ps[:, 256 + h2 * 128:256 + (h2 + 1) * 128],
                                ones_sb[0:1, :],
                                state["rcT_sb"][:, h2 * 128:(h2 + 1) * 128],
                                start=True, stop=True, skip_group_check=True)
                        nc.vector.tensor_mul(yt_sb[h][:, wsl],
                                             state["ys"][:, 0:SW],
                                             rc_ps[:, 256:512])
                    units.append(rc2)
                return units

            def weave(a_units, f_units):
                for a in a_units:
                    a()
                    if f_units:
                        f_units.pop(0)()
                for f in f_units:
                    f()

            # ---------------- main schedule ----------------
            emit_startup_dmas()
            for i in range(NSQ):
                if i + 3 < NSQ:
                    load_xt(i + 3)
                f_units = [proj_ct_unit(i, ct) for ct in range(NCT)]
                if i >= 2:
                    f_units += make_po_units(i - 2)
                a_units = make_attn_units(i - 1) if i >= 1 else []
                weave(a_units, f_units)
            # tail: window 7 + outproj windows 6, 7
            weave(make_attn_units(NSQ - 1), make_po_units(NSQ - 2))
            for u in make_po_units(NSQ - 1):
                u()

    nc.compile()
    return nc


_CACHE = {}


def _get_module():
    if "nc" not in _CACHE:
        nc = bacc.Bacc("TRN2", target_bir_lowering=False, debug=False)
        _CACHE["nc"] = _emit(nc)
    return _CACHE["nc"]


def _host_constants():
    if "consts" in _CACHE:
        return _CACHE["consts"]
    inv_freq = 1.0 / (ROPE_BASE ** (np.arange(0, DH, 2, dtype=np.float64) / DH))
    ang = np.outer(np.arange(T, dtype=np.float64), inv_freq)      # (T, 64)
    emb = np.concatenate([ang, ang], axis=-1)                     # (T, 128)
    cos = np.cos(emb).astype(np.float32)                          # (T, 128)
    sin = np.sin(emb).astype(np.float32)
    cosT = np.ascontiguousarray(cos.T).astype(BF)                 # (128, T)
    sign = np.where(np.arange(DH) < DH // 2, -1.0, 1.0).astype(np.float32)
    sinR = np.ascontiguousarray(sin.T * sign[:, None]).astype(BF)
    ones = np.ones((128, 128), dtype=BF)
    identf = np.eye(128, dtype=np.float32)
    # causal masks for the diagonal chunk pair of each 256-wide window
    j = np.arange(SW)[None, :]
    p = np.arange(128)[:, None]
    m0 = (j >= p).astype(BF)
    m1 = (j >= p + 128).astype(BF)
    masks = np.concatenate([m0, m1], axis=1)                      # (128, 512)
    _CACHE["consts"] = (cosT, sinR, ones, identf, masks)
    return _CACHE["consts"]


def kernel(x, wq, wk, wv, wproj):
    x = np.asarray(x, dtype=np.float32)
    wq = np.asarray(wq, dtype=np.float32)
    wk = np.asarray(wk, dtype=np.float32)
    wv = np.asarray(wv, dtype=np.float32)
    wproj = np.asarray(wproj, dtype=np.float32)

    nc = _get_module()
    cosT, sinR, ones, identf, masks = _host_constants()
    xT = np.ascontiguousarray(x[0].T).astype(BF)                  # (C, T)

    in_maps = []
    for d in range(N_CORES):
        h0 = HPD * d
        g = d // 2
        # wq columns for heads h0..h0+HPD-1 -> [128, NCT*HPD*DH] (c-tile major)
        wq_d = wq[:, h0 * DH:(h0 + HPD) * DH]                     # (C, HPD*DH)
        wq_l = np.ascontiguousarray(
            wq_d.reshape(NCT, 128, HPD * DH).transpose(1, 0, 2).reshape(128, -1)
        ).astype(BF)
        wk_d = wk[:, g * DH:(g + 1) * DH]
        wk_l = np.ascontiguousarray(
            wk_d.reshape(NCT, 128, DH).transpose(1, 0, 2).reshape(128, -1)
        ).astype(BF)
        wv_d = wv[:, g * DH:(g + 1) * DH]
        wv_l = np.ascontiguousarray(
            wv_d.reshape(NCT, 128, DH).transpose(1, 0, 2).reshape(128, -1)
        ).astype(BF)
        # wproj rows for our heads -> [128, HPD*C] (head-major free dim)
        wp_d = wproj[h0 * DH:(h0 + HPD) * DH, :]                  # (HPD*DH, C)
        wp_l = np.ascontiguousarray(
            wp_d.reshape(HPD, 128, C).transpose(1, 0, 2).reshape(128, -1)
        ).astype(BF)
        in_maps.append({
            "xT": xT, "wq": wq_l, "wk": wk_l, "wv": wv_l, "wp": wp_l,
            "cosT": cosT, "sinR": sinR, "ones": ones, "identf": identf,
            "masks": masks,
        })

    res = run_bass_kernel_spmd(nc, in_maps, core_ids=list(range(N_CORES)))
    acc = res.results[0]["out"].astype(np.float32)
    for d in range(1, N_CORES):
        acc = acc + res.results[d]["out"].astype(np.float32)
    return acc.reshape(1, T, C)


# revision 8
# speedup vs baseline: 1.1028x; 1.0204x over previous
"""Trainium2 Bass kernel for CausalSelfAttentionModern (GQA + RoPE + causal SDPA).

Sharding: tensor-parallel over heads across 8 NeuronCores.
Device d owns q-heads {2d, 2d+1} and kv-head d//2.
Each device computes its heads' attention plus its slice of the output
projection (row-parallel); the host sums the 8 partial outputs.

v3: bf16 data path (DMA + matmul inputs; fp32 PSUM accumulation),
softmax denominator via tiny stationary-pt matmuls instead of a full
ones-matmul, V projected directly in [token, dh] layout, causal mask via
constant-mask multiply, packed weight/constant tensors to minimize DMA
trigger count (HWDGE is 625ns/DMA serialized), and a unified PE weave:
projection of sub-quarter i and output projection of window i-2 fill the
latency bubbles of attention window i-1.
"""

import numpy as np
import ml_dtypes
import concourse.bacc as bacc
import concourse.tile as tile
import concourse.mybir as mybir
from concourse.bass_utils import run_bass_kernel_spmd

F32 = mybir.dt.float32
BF16 = mybir.dt.bfloat16
EXP = mybir.ActivationFunctionType.Exp

# hardcoded problem shapes
T = 2048          # sequence length
C = 2048          # embedding dim
DH = 128          # head dim
NH = 16           # query heads
NKV = 4           # kv heads
N_CORES = 8
HPD = NH // N_CORES  # q-heads per device = 2
ROPE_BASE = 10000.0
SCALE = 1.0 / np.sqrt(DH)

NCT = C // 128    # 16 contraction tiles
NTC = T // 128    # 16 token chunks
SW = 256          # sub-quarter / attention window width
NSQ = T // SW     # 8 windows
WCT = HPD * DH + 2 * DH  # packed weight cols per c-tile = 512

BF = ml_dtypes.bfloat16


def _emit(nc):
    xT = nc.dram_tensor("xT", [C, T], BF16, kind="ExternalInput").ap()
    # packed per c-tile: [wq(2 heads) | wk | wv] = 512 cols each
    wqkv = nc.dram_tensor("wqkv", [128, NCT * WCT], BF16, kind="ExternalInput").ap()
    wp = nc.dram_tensor("wp", [128, HPD * C], BF16, kind="ExternalInput").ap()
    trig = nc.dram_tensor("trig", [128, 2 * T], BF16, kind="ExternalInput").ap()
    miscb = nc.dram_tensor("miscb", [128, 128 + 2 * SW], BF16,
                           kind="ExternalInput").ap()
    identf = nc.dram_tensor("identf", [128, 128], F32, kind="ExternalInput").ap()
    out = nc.dram_tensor("out", [T, C], BF16, kind="ExternalOutput").ap()

    with tile.TileContext(nc) as tc:
        with (
            tc.tile_pool(name="cst", bufs=1) as cst,
            tc.tile_pool(name="ps", bufs=1, space="PSUM") as ps,
            tc.tile_pool(name="xts", bufs=4) as xts,
            tc.tile_pool(name="rope", bufs=4) as rope,
            tc.tile_pool(name="pts", bufs=12) as pts,
            tc.tile_pool(name="osts", bufs=3) as osts,
            tc.tile_pool(name="rcs", bufs=2) as rcs,
        ):
            # persistent SBUF tensors
            trig_sb = cst.tile([128, 2 * T], BF16, tag="trig")
            cos_sb = trig_sb[:, 0:T]
            sin_sb = trig_sb[:, T:2 * T]
            misc_sb = cst.tile([128, 128 + 2 * SW], BF16, tag="miscb")
            ones_sb = misc_sb[:, 0:128]
            masks_sb = misc_sb[:, 128:128 + 2 * SW]
            identf_sb = cst.tile([128, 128], F32, tag="identf")
            scratch = cst.tile([128, 256], BF16, tag="scratch")
            wqkv_sb = cst.tile([128, NCT * WCT], BF16, tag="wqkv")
            wp_sb = cst.tile([128, HPD * C], BF16, tag="wp")

            qt_sb = [cst.tile([128, T], BF16, tag=f"qt{m}", name=f"qt{m}")
                     for m in range(HPD)]
            kt_sb = cst.tile([128, T], BF16, tag="kt")
            v_sb = cst.tile([128, NTC * DH], BF16, tag="v")
            yt_sb = [cst.tile([128, T], BF16, tag=f"yt{m}", name=f"yt{m}")
                     for m in range(HPD)]

            def wq_ap(ct, m):
                return wqkv_sb[:, ct * WCT + m * DH:ct * WCT + (m + 1) * DH]

            def wk_ap(ct):
                return wqkv_sb[:, ct * WCT + HPD * DH:ct * WCT + HPD * DH + DH]

            def wv_ap(ct):
                return wqkv_sb[:, ct * WCT + HPD * DH + DH:(ct + 1) * WCT]

            # ---- PE warmup: ramp the p-state before real work arrives ----
            nc.vector.memset(scratch[:], 0)
            warm_ps = ps.tile([128, 512], F32, tag="sc", bufs=2, name="warm")
            for wdx in range(12):
                nc.tensor.matmul(warm_ps[:, 0:256], scratch[:, 0:128], scratch[:],
                                 start=True, stop=True, skip_group_check=True)

            # ---- DMA emission helpers ----
            xt_tiles = {}

            def emit_xt_dma(i, a, b):
                tsl = slice(i * SW, (i + 1) * SW)
                xt = xt_tiles[i]
                nc.sync.dma_start(
                    xt[:, a * SW:b * SW].rearrange("p (ct t) -> p ct t", t=SW),
                    xT[a * 128:b * 128, tsl].rearrange("(ct p) t -> p ct t", p=128))

            def load_xt(i, split=False):
                xt_tiles[i] = xts.tile([128, NCT * SW], BF16, tag="xt",
                                       name=f"xt{i}")
                if split:
                    emit_xt_dma(i, 0, 3)
                else:
                    emit_xt_dma(i, 0, 16)

            def emit_startup_dmas():
                # first x slice + first weight c-tiles unblock proj(0) asap
                load_xt(0, split=True)
                nc.sync.dma_start(wqkv_sb[:, 0:2 * WCT], wqkv[:, 0:2 * WCT])
                emit_xt_dma(0, 3, 8)
                nc.sync.dma_start(wqkv_sb[:, 2 * WCT:9 * WCT],
                                  wqkv[:, 2 * WCT:9 * WCT])
                emit_xt_dma(0, 8, 16)
                nc.sync.dma_start(wqkv_sb[:, 9 * WCT:16 * WCT],
                                  wqkv[:, 9 * WCT:16 * WCT])
                # constants on the ACT ring in parallel
                nc.scalar.dma_start(trig_sb[:], trig[:])
                nc.scalar.dma_start(misc_sb[:], miscb[:])
                nc.scalar.dma_start(identf_sb[:], identf[:])
                nc.scalar.dma_start(wp_sb[:], wp[:])
                load_xt(1)
                load_xt(2)

            # ---- projection units (PE filler) ----
            proj_psum = {}

            def proj_ct_unit(i, ct):
                def unit():
                    if ct == 0:
                        proj_psum[i] = (
                            ps.tile([128, 512], F32, tag="pqq", bufs=1,
                                    name=f"pqq{i}"),
                            ps.tile([128, 512], F32, tag="pkv", bufs=1,
                                    name=f"pkv{i}"),
                        )
                    pqq, pkv = proj_psum[i]
                    xt = xt_tiles[i]
                    xsl = slice(ct * SW, (ct + 1) * SW)
                    st = ct == 0
                    sp = ct == NCT - 1
                    for m in range(HPD):
                        nc.tensor.matmul(
                            pqq[:, m * SW:(m + 1) * SW], wq_ap(ct, m),
                            xt[:, xsl], start=st, stop=sp, skip_group_check=True)
                    nc.tensor.matmul(
                        pkv[:, 0:SW], wk_ap(ct),
                        xt[:, xsl], start=st, stop=sp, skip_group_check=True)
                    # v in [token, dh] layout: x chunk stationary, wv moving
                    for j in range(2):
                        nc.tensor.matmul(
                            pkv[:, SW + j * DH:SW + (j + 1) * DH],
                            xt[:, ct * SW + j * 128:ct * SW + (j + 1) * 128],
                            wv_ap(ct), start=st, stop=sp, skip_group_check=True)
                    if sp:
                        emit_rope_v(i)
                return unit

            def emit_rope_v(i):
                # rope: copy psum->sbuf bf16 (Pool), then 2x-mode DVE ops
                tsl = slice(i * SW, (i + 1) * SW)
                pqq, pkv = proj_psum[i]
                for idx, dst in [(0, qt_sb[0]), (1, qt_sb[1]), (2, kt_sb)]:
                    src = pkv if dst is kt_sb else pqq
                    psl = slice(0, SW) if idx != 1 else slice(SW, 2 * SW)
                    p_sb = rope.tile([128, SW], BF16, tag="p_sb")
                    nc.gpsimd.tensor_copy(p_sb[:], src[:, psl])
                    cr = rope.tile([128, SW], BF16, tag="cr")
                    nc.vector.tensor_mul(cr[:], p_sb[:], cos_sb[:, tsl])
                    ur = rope.tile([128, SW], BF16, tag="ur")
                    nc.vector.tensor_mul(ur[0:64, :], p_sb[64:128, :],
                                         sin_sb[0:64, tsl])
                    nc.vector.tensor_mul(ur[64:128, :], p_sb[0:64, :],
                                         sin_sb[64:128, tsl])
                    nc.vector.tensor_add(dst[:, tsl], cr[:], ur[:])
                # v: plain copies to [token, dh] sbuf
                for j in range(2):
                    tc0 = 2 * i + j
                    nc.gpsimd.tensor_copy(v_sb[:, tc0 * DH:(tc0 + 1) * DH],
                                          pkv[:, SW + j * DH:SW + (j + 1) * DH])

            # ---- output projection units (PE filler) ----
            OST_ENG = [nc.scalar.copy, nc.vector.tensor_copy,
                       nc.gpsimd.tensor_copy, nc.vector.tensor_copy]

            def make_po_units(w):
                units = []
                for j in range(2):
                    t0 = w * SW + j * 128
                    ost = osts.tile([128, C], BF16, tag="ost", name=f"ost{t0}")
                    for e in range(4):
                        def unit(t0=t0, ost=ost, e=e):
                            po = ps.tile([128, 512], F32, tag="po", bufs=2,
                                         name=f"po{t0}_{e}")
                            for k in range(HPD):
                                nc.tensor.matmul(
                                    po[:],
                                    yt_sb[k][:, t0:t0 + 128],
                                    wp_sb[:, k * C + e * 512:k * C + (e + 1) * 512],
                                    start=(k == 0), stop=(k == HPD - 1))
                            OST_ENG[e](ost[:, e * 512:(e + 1) * 512], po[:])
                        units.append(unit)

                    def dma_unit(t0=t0, ost=ost):
                        nc.scalar.dma_start(out[t0:t0 + 128, :], ost[:])
                    units.append(dma_unit)
                return units

            # ---- attention units ----
            def make_attn_units(w):
                """Window w, both heads: scores/exp/mask, y/s, rc chain."""
                units = []
                npair = w + 1
                tw0 = w * SW
                wsl = slice(tw0, tw0 + SW)

                def y_pair(h, cp, state):
                    nch = 2 * npair
                    ys = state["ys"]
                    pt = state["pts"][cp]
                    for k2 in range(2):
                        cc = 2 * cp + k2
                        st = cc == 0
                        sp = cc == nch - 1
                        psl = slice(k2 * SW, (k2 + 1) * SW)
                        nc.tensor.matmul(
                            ys[:, 0:SW], v_sb[:, cc * DH:(cc + 1) * DH],
                            pt[:, psl], start=st, stop=sp,
                            skip_group_check=True)
                        for h2 in range(2):
                            nc.tensor.matmul(
                                ys[:, SW + h2:SW + h2 + 1],
                                pt[:, k2 * SW + h2 * 128:k2 * SW + (h2 + 1) * 128],
                                ones_sb[:, 0:1], start=st, stop=sp,
                                skip_group_check=True)

                def sc_unit(h, cp, state):
                    if cp == 0:
                        state["ys"] = ps.tile([128, 512], F32, tag="ys",
                                              bufs=2, name=f"ys{w}_{h}")
                    sc_ps = ps.tile([128, 512], F32, tag="sc", bufs=2,
                                    name=f"sc{w}_{h}_{cp}")
                    for k2 in range(2):
                        cc = 2 * cp + k2
                        nc.tensor.matmul(
                            sc_ps[:, k2 * SW:(k2 + 1) * SW],
                            kt_sb[:, cc * 128:(cc + 1) * 128],
                            qt_sb[h][:, wsl], start=True, stop=True)
                    pt = pts.tile([128, 512], BF16, tag="pt",
                                  name=f"pt{w}_{h}_{cp}")
                    nc.scalar.activation(pt[:], sc_ps[:], EXP,
                                         scale=float(SCALE))
                    if cp == npair - 1:
                        # diagonal pair: zero strictly-above-diagonal
                        nc.vector.tensor_mul(pt[:], pt[:], masks_sb[:])
                    state["pts"][cp] = pt
                    if cp >= 1:
                        y_pair(h, cp - 1, state)

                for h in range(HPD):
                    state = {"pts": {}, "ys": None}

                    for cp in range(npair):
                        units.append(lambda h=h, cp=cp, state=state:
                                     sc_unit(h, cp, state))

                    def y_last(h=h, state=state):
                        y_pair(h, npair - 1, state)
                        s_sb = rcs.tile([128, 2], F32, tag="s_sb",
                                        name=f"ssb{w}_{h}")
                        nc.vector.tensor_copy(s_sb[:], state["ys"][:, SW:SW + 2])
                        state["s_sb"] = s_sb
                    units.append(y_last)

                    def rc1(h=h, state=state):
                        rc_ps = ps.tile([128, 512], F32, tag="sc", bufs=2,
                                        name=f"rcps{w}_{h}")
                        for h2 in range(2):
                            nc.tensor.transpose(
                                rc_ps[0:1, h2 * 128:(h2 + 1) * 128],
                                state["s_sb"][:, h2:h2 + 1], identf_sb[:])
                        state["rc_ps"] = rc_ps
                        rcT_sb = rcs.tile([1, 256], BF16, tag="rcT",
                                          name=f"rcT{w}_{h}")
                        with nc.allow_low_precision("softmax denominators"):
                            nc.vector.reciprocal(rcT_sb[:], rc_ps[0:1, 0:256])
                        state["rcT_sb"] = rcT_sb
                    units.append(rc1)

                    def rc2(h=h, state=state):
                        rc_ps = state["rc_ps"]
                        for h2 in range(2):
                            nc.tensor.matmul(
                                rc_ps[:, 256 + h2 * 128:256 + (h2 + 1) * 128],
                                ones_sb[0:1, :],
                                state["rcT_sb"][:, h2 * 128:(h2 + 1) * 128],
                                start=True, stop=True, skip_group_check=True)
                        nc.vector.tensor_mul(yt_sb[h][:, wsl],
                                             state["ys"][:, 0:SW],
                                             rc_ps[:, 256:512])
                    units.append(rc2)
                return units

            def weave(a_units, f_units):
                # distribute fillers evenly across the attention stream
                na, nf = len(a_units), len(f_units)
                if na == 0:
                    for f in f_units:
                        f()
                    return
                taken = 0
                for idx, a in enumerate(a_units):
                    a()
                    want = min(nf, ((idx + 1) * nf) // na)
                    while taken < want:
                        f_units[taken]()
                        taken += 1

            # ---------------- main schedule ----------------
            emit_startup_dmas()
            for i in range(NSQ):
                if i + 3 < NSQ:
                    load_xt(i + 3)
                f_units = [proj_ct_unit(i, ct) for ct in range(NCT)]
                if i >= 2:
                    f_units += make_po_units(i - 2)
                a_units = make_attn_units(i - 1) if i >= 1 else []
                weave(a_units, f_units)
            # tail: window 7 + outproj windows 6, 7
            weave(make_attn_units(NSQ - 1), make_po_units(NSQ - 2))
            for u in make_po_units(NSQ - 1):
                u()

    nc.compile()
    return nc


_CACHE = {}


def _get_module():
    if "nc" not in _CACHE:
        nc = bacc.Bacc("TRN2", target_bir_lowering=False, debug=False)
        _CACHE["nc"] = _emit(nc)
    return _CACHE["nc"]


def _host_constants():
    if "consts" in _CACHE:
        return _CACHE["consts"]
    inv_freq = 1.0 / (ROPE_BASE ** (np.arange(0, DH, 2, dtype=np.float64) / DH))
    ang = np.outer(np.arange(T, dtype=np.float64), inv_freq)      # (T, 64)
    emb = np.concatenate([ang, ang], axis=-1)                     # (T, 128)
    cos = np.cos(emb).astype(np.float32)                          # (T, 128)
    sin = np.sin(emb).astype(np.float32)
    cosT = np.ascontiguousarray(cos.T)                            # (128, T)
    sign = np.where(np.arange(DH) < DH // 2, -1.0, 1.0).astype(np.float32)
    sinR = np.ascontiguousarray(sin.T * sign[:, None])
    trig = np.concatenate([cosT, sinR], axis=1).astype(BF)        # (128, 2T)
    ones = np.ones((128, 128), dtype=np.float32)
    identf = np.eye(128, dtype=np.float32)
    # causal masks for the diagonal chunk pair of each 256-wide window
    j = np.arange(SW)[None, :]
    p = np.arange(128)[:, None]
    m0 = (j >= p).astype(np.float32)
    m1 = (j >= p + 128).astype(np.float32)
    miscb = np.concatenate([ones, m0, m1], axis=1).astype(BF)     # (128, 640)
    _CACHE["consts"] = (trig, miscb, identf)
    return _CACHE["consts"]


def kernel(x, wq, wk, wv, wproj):
    x = np.asarray(x, dtype=np.float32)
    wq = np.asarray(wq, dtype=np.float32)
    wk = np.asarray(wk, dtype=np.float32)
    wv = np.asarray(wv, dtype=np.float32)
    wproj = np.asarray(wproj, dtype=np.float32)

    nc = _get_module()
    trig, miscb, identf = _host_constants()
    xT = np.ascontiguousarray(x[0].T).astype(BF)                  # (C, T)

    in_maps = []
    for d in range(N_CORES):
        h0 = HPD * d
        g = d // 2
        # packed weights per c-tile: [wq(2 heads) | wk | wv]
        wq_d = wq[:, h0 * DH:(h0 + HPD) * DH].reshape(NCT, 128, HPD * DH)
        wk_d = wk[:, g * DH:(g + 1) * DH].reshape(NCT, 128, DH)
        wv_d = wv[:, g * DH:(g + 1) * DH].reshape(NCT, 128, DH)
        wqkv_d = np.concatenate([wq_d, wk_d, wv_d], axis=2)       # (NCT,128,512)
        wqkv_l = np.ascontiguousarray(
            wqkv_d.transpose(1, 0, 2).reshape(128, -1)).astype(BF)
        # wproj rows for our heads -> [128, HPD*C] (head-major free dim)
        wp_d = wproj[h0 * DH:(h0 + HPD) * DH, :]                  # (HPD*DH, C)
        wp_l = np.ascontiguousarray(
            wp_d.reshape(HPD, 128, C).transpose(1, 0, 2).reshape(128, -1)
        ).astype(BF)
        in_maps.append({
            "xT": xT, "wqkv": wqkv_l, "wp": wp_l,
            "trig": trig, "miscb": miscb, "identf": identf,
        })

    res = run_bass_kernel_spmd(nc, in_maps, core_ids=list(range(N_CORES)))
    acc = res.results[0]["out"].astype(np.float32)
    for d in range(1, N_CORES):
        acc = acc + res.results[d]["out"].astype(np.float32)
    return acc.reshape(1, T, C)


# revision 14
# speedup vs baseline: 1.1718x; 1.0626x over previous
"""Trainium2 Bass kernel for CausalSelfAttentionModern (GQA + RoPE + causal SDPA).

Sharding: tensor-parallel over heads across 8 NeuronCores.
Device d owns q-heads {2d, 2d+1} and kv-head d//2.
Each device computes its heads' attention plus its slice of the output
projection (row-parallel); the host sums the 8 partial outputs.

v3: bf16 data path (DMA + matmul inputs; fp32 PSUM accumulation),
softmax denominator via tiny stationary-pt matmuls instead of a full
ones-matmul, V projected directly in [token, dh] layout, causal mask via
constant-mask multiply, packed weight/constant tensors to minimize DMA
trigger count (HWDGE is 625ns/DMA serialized), and a unified PE weave:
projection of sub-quarter i and output projection of window i-2 fill the
latency bubbles of attention window i-1.
"""

import numpy as np
import ml_dtypes
import concourse.bacc as bacc
import concourse.tile as tile
import concourse.mybir as mybir
from concourse.bass_utils import run_bass_kernel_spmd

F32 = mybir.dt.float32
BF16 = mybir.dt.bfloat16
EXP = mybir.ActivationFunctionType.Exp

# hardcoded problem shapes
T = 2048          # sequence length
C = 2048          # embedding dim
DH = 128          # head dim
NH = 16           # query heads
NKV = 4           # kv heads
N_CORES = 8
HPD = NH // N_CORES  # q-heads per device = 2
ROPE_BASE = 10000.0
SCALE = 1.0 / np.sqrt(DH)

NCT = C // 128    # 16 contraction tiles
NTC = T // 128    # 16 token chunks
SW = 256          # sub-quarter / attention window width
NSQ = T // SW     # 8 windows
WCT = HPD * DH + 2 * DH  # packed weight cols per c-tile = 512

BF = ml_dtypes.bfloat16


def _emit(nc):
    xT = nc.dram_tensor("xT", [C, T], BF16, kind="ExternalInput").ap()
    # packed per c-tile: [wq(2 heads) | wk | wv] = 512 cols each
    wqkv = nc.dram_tensor("wqkv", [128, NCT * WCT], BF16, kind="ExternalInput").ap()
    wp = nc.dram_tensor("wp", [128, HPD * C], BF16, kind="ExternalInput").ap()
    trig = nc.dram_tensor("trig", [128, 2 * T], BF16, kind="ExternalInput").ap()
    miscb = nc.dram_tensor("miscb", [128, 128 + 2 * SW], BF16,
                           kind="ExternalInput").ap()
    identf = nc.dram_tensor("identf", [128, 128], F32, kind="ExternalInput").ap()
    out = nc.dram_tensor("out", [T, C], BF16, kind="ExternalOutput").ap()

    with tile.TileContext(nc) as tc:
        with (
            tc.tile_pool(name="cst", bufs=1) as cst,
            tc.tile_pool(name="ps", bufs=1, space="PSUM") as ps,
            tc.tile_pool(name="xts", bufs=4) as xts,
            tc.tile_pool(name="rope", bufs=4) as rope,
            tc.tile_pool(name="pts", bufs=12) as pts,
            tc.tile_pool(name="osts", bufs=3) as osts,
            tc.tile_pool(name="rcs", bufs=2) as rcs,
        ):
            # persistent SBUF tensors
            trig_sb = cst.tile([128, 2 * T], BF16, tag="trig")
            cos_sb = trig_sb[:, 0:T]
            sin_sb = trig_sb[:, T:2 * T]
            misc_sb = cst.tile([128, 128 + 2 * SW], BF16, tag="miscb")
            ones_sb = misc_sb[:, 0:128]
            masks_sb = misc_sb[:, 128:128 + 2 * SW]
            identf_sb = cst.tile([128, 128], F32, tag="identf")
            scratch = cst.tile([128, 256], BF16, tag="scratch")
            wqkv_sb = cst.tile([128, NCT * WCT], BF16, tag="wqkv")
            wp_sb = cst.tile([128, HPD * C], BF16, tag="wp")

            qt_sb = [cst.tile([128, T], BF16, tag=f"qt{m}", name=f"qt{m}")
                     for m in range(HPD)]
            kt_sb = cst.tile([128, T], BF16, tag="kt")
            v_sb = cst.tile([128, NTC * DH], BF16, tag="v")
            yt_sb = [cst.tile([128, T], BF16, tag=f"yt{m}", name=f"yt{m}")
                     for m in range(HPD)]

            def wq_ap(ct, m):
                return wqkv_sb[:, ct * WCT + m * DH:ct * WCT + (m + 1) * DH]

            def wk_ap(ct):
                return wqkv_sb[:, ct * WCT + HPD * DH:ct * WCT + HPD * DH + DH]

            def wv_ap(ct):
                return wqkv_sb[:, ct * WCT + HPD * DH + DH:(ct + 1) * WCT]

            # ---- PE warmup: ramp the p-state before real work arrives ----
            nc.vector.memset(scratch[:], 0)
            warm_ps = ps.tile([128, 512], F32, tag="sc", bufs=2, name="warm")
            for wdx in range(12):
                nc.tensor.matmul(warm_ps[:, 0:256], scratch[:, 0:128], scratch[:],
                                 start=True, stop=True, skip_group_check=True)

            # ---- DMA emission helpers ----
            xt_tiles = {}

            def emit_xt_dma(i, a, b):
                tsl = slice(i * SW, (i + 1) * SW)
                xt = xt_tiles[i]
                nc.sync.dma_start(
                    xt[:, a * SW:b * SW].rearrange("p (ct t) -> p ct t", t=SW),
                    xT[a * 128:b * 128, tsl].rearrange("(ct p) t -> p ct t", p=128))

            def load_xt(i, split=False):
                xt_tiles[i] = xts.tile([128, NCT * SW], BF16, tag="xt",
                                       name=f"xt{i}")
                if split:
                    emit_xt_dma(i, 0, 3)
                else:
                    emit_xt_dma(i, 0, 16)

            def emit_startup_dmas():
                # single sync-ring stream ordered by first use; HWDGE issues
                # one DMA per ~625ns so order is everything here
                load_xt(0, split=True)
                nc.sync.dma_start(wqkv_sb[:, 0:2 * WCT], wqkv[:, 0:2 * WCT])
                emit_xt_dma(0, 3, 8)
                nc.sync.dma_start(wqkv_sb[:, 2 * WCT:9 * WCT],
                                  wqkv[:, 2 * WCT:9 * WCT])
                emit_xt_dma(0, 8, 16)
                nc.sync.dma_start(trig_sb[:, 0:T], trig[:, 0:T])
                nc.sync.dma_start(wqkv_sb[:, 9 * WCT:16 * WCT],
                                  wqkv[:, 9 * WCT:16 * WCT])
                nc.sync.dma_start(trig_sb[:, T:2 * T], trig[:, T:2 * T])
                load_xt(1)
                nc.sync.dma_start(misc_sb[:], miscb[:])
                nc.sync.dma_start(identf_sb[:], identf[:])
                load_xt(2)
                nc.sync.dma_start(wp_sb[:], wp[:])

            # ---- projection units (PE filler) ----
            proj_psum = {}

            def proj_ct_unit(i, ct):
                def unit():
                    if ct == 0:
                        proj_psum[i] = (
                            ps.tile([128, 512], F32, tag="pqq", bufs=1,
                                    name=f"pqq{i}"),
                            ps.tile([128, 512], F32, tag="pkv", bufs=1,
                                    name=f"pkv{i}"),
                        )
                    pqq, pkv = proj_psum[i]
                    xt = xt_tiles[i]
                    xsl = slice(ct * SW, (ct + 1) * SW)
                    st = ct == 0
                    sp = ct == NCT - 1
                    for m in range(HPD):
                        nc.tensor.matmul(
                            pqq[:, m * SW:(m + 1) * SW], wq_ap(ct, m),
                            xt[:, xsl], start=st, stop=sp, skip_group_check=True)
                    nc.tensor.matmul(
                        pkv[:, 0:SW], wk_ap(ct),
                        xt[:, xsl], start=st, stop=sp, skip_group_check=True)
                    # v in [token, dh] layout: x chunk stationary, wv moving
                    for j in range(2):
                        nc.tensor.matmul(
                            pkv[:, SW + j * DH:SW + (j + 1) * DH],
                            xt[:, ct * SW + j * 128:ct * SW + (j + 1) * 128],
                            wv_ap(ct), start=st, stop=sp, skip_group_check=True)
                    if sp:
                        emit_rope_v(i)
                return unit

            def emit_rope_v(i):
                # rope: copy psum->sbuf bf16 (Pool), then 2x-mode DVE ops
                tsl = slice(i * SW, (i + 1) * SW)
                pqq, pkv = proj_psum[i]
                for idx, dst in [(0, qt_sb[0]), (1, qt_sb[1]), (2, kt_sb)]:
                    src = pkv if dst is kt_sb else pqq
                    psl = slice(0, SW) if idx != 1 else slice(SW, 2 * SW)
                    p_sb = rope.tile([128, SW], BF16, tag="p_sb")
                    nc.gpsimd.tensor_copy(p_sb[:], src[:, psl])
                    cr = rope.tile([128, SW], BF16, tag="cr")
                    nc.vector.tensor_mul(cr[:], p_sb[:], cos_sb[:, tsl])
                    ur = rope.tile([128, SW], BF16, tag="ur")
                    nc.vector.tensor_mul(ur[0:64, :], p_sb[64:128, :],
                                         sin_sb[0:64, tsl])
                    nc.vector.tensor_mul(ur[64:128, :], p_sb[0:64, :],
                                         sin_sb[64:128, tsl])
                    nc.vector.tensor_add(dst[:, tsl], cr[:], ur[:])
                # v: plain copies to [token, dh] sbuf
                for j in range(2):
                    tc0 = 2 * i + j
                    nc.gpsimd.tensor_copy(v_sb[:, tc0 * DH:(tc0 + 1) * DH],
                                          pkv[:, SW + j * DH:SW + (j + 1) * DH])

            # ---- output projection units (PE filler) ----
            OST_ENG = [nc.scalar.copy, nc.vector.tensor_copy,
                       nc.gpsimd.tensor_copy, nc.vector.tensor_copy]

            PS_BUFS = {"po": 2, "sc": 2, "ys": 2, "pqq": 1, "pkv": 1}

            def make_po_units(w, tag_cycle=("po",)):
                units = []
                uidx = [0]
                for j in range(2):
                    t0 = w * SW + j * 128
                    ost = osts.tile([128, C], BF16, tag="ost", name=f"ost{t0}")
                    for e in range(4):
                        def unit(t0=t0, ost=ost, e=e):
                            tg = tag_cycle[uidx[0] % len(tag_cycle)]
                            uidx[0] += 1
                            po = ps.tile([128, 512], F32, tag=tg,
                                         bufs=PS_BUFS[tg], name=f"po{t0}_{e}")
                            for k in range(HPD):
                                nc.tensor.matmul(
                                    po[:],
                                    yt_sb[k][:, t0:t0 + 128],
                                    wp_sb[:, k * C + e * 512:k * C + (e + 1) * 512],
                                    start=(k == 0), stop=(k == HPD - 1))
                            OST_ENG[e](ost[:, e * 512:(e + 1) * 512], po[:])
                        units.append(unit)

                    def dma_unit(t0=t0, ost=ost):
                        nc.scalar.dma_start(out[t0:t0 + 128, :], ost[:])
                    units.append(dma_unit)
                return units

            # ---- attention units ----
            def make_attn_units(w, sc_tags=("sc",)):
                """Window w, both heads: scores/exp/mask, y/s, rc chain."""
                units = []
                npair = w + 1
                tw0 = w * SW
                wsl = slice(tw0, tw0 + SW)
                scidx = [0]

                def sc_tile(name):
                    tg = sc_tags[scidx[0] % len(sc_tags)]
                    scidx[0] += 1
                    return ps.tile([128, 512], F32, tag=tg, bufs=PS_BUFS[tg],
                                   name=name)

                def y_pair(h, cp, state):
                    nch = 2 * npair
                    ys = state["ys"]
                    pt = state["pts"][cp]
                    for k2 in range(2):
                        cc = 2 * cp + k2
                        st = cc == 0
                        sp = cc == nch - 1
                        psl = slice(k2 * SW, (k2 + 1) * SW)
                        nc.tensor.matmul(
                            ys[:, 0:SW], v_sb[:, cc * DH:(cc + 1) * DH],
                            pt[:, psl], start=st, stop=sp,
                            skip_group_check=True)
                        for h2 in range(2):
                            nc.tensor.matmul(
                                ys[:, SW + h2:SW + h2 + 1],
                                pt[:, k2 * SW + h2 * 128:k2 * SW + (h2 + 1) * 128],
                                ones_sb[:, 0:1], start=st, stop=sp,
                                skip_group_check=True)

                def sc_unit(h, cp, state):
                    if cp == 0:
                        state["ys"] = ps.tile([128, 512], F32, tag="ys",
                                              bufs=2, name=f"ys{w}_{h}")
                    sc_ps = sc_tile(f"sc{w}_{h}_{cp}")
                    for k2 in range(2):
                        cc = 2 * cp + k2
                        nc.tensor.matmul(
                            sc_ps[:, k2 * SW:(k2 + 1) * SW],
                            kt_sb[:, cc * 128:(cc + 1) * 128],
                            qt_sb[h][:, wsl], start=True, stop=True)
                    pt = pts.tile([128, 512], BF16, tag="pt",
                                  name=f"pt{w}_{h}_{cp}")
                    nc.scalar.activation(pt[:], sc_ps[:], EXP,
                                         scale=float(SCALE))
                    if cp == npair - 1:
                        # diagonal pair: zero strictly-above-diagonal
                        nc.vector.tensor_mul(pt[:], pt[:], masks_sb[:])
                    state["pts"][cp] = pt
                    if cp >= 1:
                        y_pair(h, cp - 1, state)

                for h in range(HPD):
                    state = {"pts": {}, "ys": None}

                    for cp in range(npair):
                        units.append(lambda h=h, cp=cp, state=state:
                                     sc_unit(h, cp, state))

                    def y_last(h=h, state=state):
                        y_pair(h, npair - 1, state)
                        s_sb = rcs.tile([128, 2], F32, tag="s_sb",
                                        name=f"ssb{w}_{h}")
                        nc.vector.tensor_copy(s_sb[:], state["ys"][:, SW:SW + 2])
                        state["s_sb"] = s_sb
                    units.append(y_last)

                    def rc1(h=h, state=state):
                        rc_ps = sc_tile(f"rcps{w}_{h}")
                        for h2 in range(2):
                            nc.tensor.transpose(
                                rc_ps[0:1, h2 * 128:(h2 + 1) * 128],
                                state["s_sb"][:, h2:h2 + 1], identf_sb[:])
                        state["rc_ps"] = rc_ps
                        rcT_sb = rcs.tile([1, 256], BF16, tag="rcT",
                                          name=f"rcT{w}_{h}")
                        with nc.allow_low_precision("softmax denominators"):
                            nc.vector.reciprocal(rcT_sb[:], rc_ps[0:1, 0:256])
                        state["rcT_sb"] = rcT_sb
                    units.append(rc1)

                    def rc2(h=h, state=state):
                        rc_ps = state["rc_ps"]
                        for h2 in range(2):
                            nc.tensor.matmul(
                                rc_ps[:, 256 + h2 * 128:256 + (h2 + 1) * 128],
                                ones_sb[0:1, :],
                                state["rcT_sb"][:, h2 * 128:(h2 + 1) * 128],
                                start=True, stop=True, skip_group_check=True)
                        nc.vector.tensor_mul(yt_sb[h][:, wsl],
                                             state["ys"][:, 0:SW],
                                             rc_ps[:, 256:512])
                    units.append(rc2)
                return units

            def weave(a_units, f_units):
                # distribute fillers evenly across the attention stream
                na, nf = len(a_units), len(f_units)
                if na == 0:
                    for f in f_units:
                        f()
                    return
                taken = 0
                for idx, a in enumerate(a_units):
                    a()
                    want = min(nf, ((idx + 1) * nf) // na)
                    while taken < want:
                        f_units[taken]()
                        taken += 1

            # ---------------- main schedule ----------------
            emit_startup_dmas()
            for i in range(NSQ):
                if i + 3 < NSQ:
                    load_xt(i + 3)
                f_units = [proj_ct_unit(i, ct) for ct in range(NCT)]
                if i >= 2:
                    f_units += make_po_units(i - 2)
                a_units = make_attn_units(i - 1) if i >= 1 else []
                weave(a_units, f_units)
            # tail: window 7 + outproj windows 6, 7; pqq/pkv banks are dead
            # here so attention scores and the last outproj rotate over them
            weave(make_attn_units(NSQ - 1, sc_tags=("sc", "pqq", "pkv")),
                  make_po_units(NSQ - 2))
            for u in make_po_units(NSQ - 1, tag_cycle=("po", "sc", "ys", "pqq")):
                u()

    nc.compile()
    return nc


_CACHE = {}


def _get_module():
    if "nc" not in _CACHE:
        nc = bacc.Bacc("TRN2", target_bir_lowering=False, debug=False)
        _CACHE["nc"] = _emit(nc)
    return _CACHE["nc"]


def _host_constants():
    if "consts" in _CACHE:
        return _CACHE["consts"]
    inv_freq = 1.0 / (ROPE_BASE ** (np.arange(0, DH, 2, dtype=np.float64) / DH))
    ang = np.outer(np.arange(T, dtype=np.float64), inv_freq)      # (T, 64)
    emb = np.concatenate([ang, ang], axis=-1)                     # (T, 128)
    cos = np.cos(emb).astype(np.float32)                          # (T, 128)
    sin = np.sin(emb).astype(np.float32)
    cosT = np.ascontiguousarray(cos.T)                            # (128, T)
    sign = np.where(np.arange(DH) < DH // 2, -1.0, 1.0).astype(np.float32)
    sinR = np.ascontiguousarray(sin.T * sign[:, None])
    trig = np.concatenate([cosT, sinR], axis=1).astype(BF)        # (128, 2T)
    ones = np.ones((128, 128), dtype=np.float32)
    identf = np.eye(128, dtype=np.float32)
    # causal masks for the diagonal chunk pair of each 256-wide window
    j = np.arange(SW)[None, :]
    p = np.arange(128)[:, None]
    m0 = (j >= p).astype(np.float32)
    m1 = (j >= p + 128).astype(np.float32)
    miscb = np.concatenate([ones, m0, m1], axis=1).astype(BF)     # (128, 640)
    _CACHE["consts"] = (trig, miscb, identf)
    return _CACHE["consts"]


def kernel(x, wq, wk, wv, wproj):
    x = np.asarray(x, dtype=np.float32)
    wq = np.asarray(wq, dtype=np.float32)
    wk = np.asarray(wk, dtype=np.float32)
    wv = np.asarray(wv, dtype=np.float32)
    wproj = np.asarray(wproj, dtype=np.float32)

    nc = _get_module()
    trig, miscb, identf = _host_constants()
    xT = np.ascontiguousarray(x[0].T).astype(BF)                  # (C, T)

    in_maps = []
    for d in range(N_CORES):
        h0 = HPD * d
        g = d // 2
        # packed weights per c-tile: [wq(2 heads) | wk | wv]
        wq_d = wq[:, h0 * DH:(h0 + HPD) * DH].reshape(NCT, 128, HPD * DH)
        wk_d = wk[:, g * DH:(g + 1) * DH].reshape(NCT, 128, DH)
        wv_d = wv[:, g * DH:(g + 1) * DH].reshape(NCT, 128, DH)
        wqkv_d = np.concatenate([wq_d, wk_d, wv_d], axis=2)       # (NCT,128,512)
        wqkv_l = np.ascontiguousarray(
            wqkv_d.transpose(1, 0, 2).reshape(128, -1)).astype(BF)
        # wproj rows for our heads -> [128, HPD*C] (head-major free dim)
        wp_d = wproj[h0 * DH:(h0 + HPD) * DH, :]                  # (HPD*DH, C)
        wp_l = np.ascontiguousarray(
            wp_d.reshape(HPD, 128, C).transpose(1, 0, 2).reshape(128, -1)
        ).astype(BF)
        in_maps.append({
            "xT": xT, "wqkv": wqkv_l, "wp": wp_l,
            "trig": trig, "miscb": miscb, "identf": identf,
        })

    res = run_bass_kernel_spmd(nc, in_maps, core_ids=list(range(N_CORES)))
    acc = res.results[0]["out"].astype(np.float32)
    for d in range(1, N_CORES):
        acc = acc + res.results[d]["out"].astype(np.float32)
    return acc.reshape(1, T, C)


# revision 19
# speedup vs baseline: 1.1771x; 1.0045x over previous
"""Trainium2 Bass kernel for CausalSelfAttentionModern (GQA + RoPE + causal SDPA).

Sharding: tensor-parallel over heads across 8 NeuronCores.
Device d owns q-heads {2d, 2d+1} and kv-head d//2.
Each device computes its heads' attention plus its slice of the output
projection (row-parallel); the host sums the 8 partial outputs.

v3: bf16 data path (DMA + matmul inputs; fp32 PSUM accumulation),
softmax denominator via tiny stationary-pt matmuls instead of a full
ones-matmul, V projected directly in [token, dh] layout, causal mask via
constant-mask multiply, packed weight/constant tensors to minimize DMA
trigger count (HWDGE is 625ns/DMA serialized), and a unified PE weave:
projection of sub-quarter i and output projection of window i-2 fill the
latency bubbles of attention window i-1.
"""

import numpy as np
import ml_dtypes
import concourse.bacc as bacc
import concourse.tile as tile
import concourse.mybir as mybir
from concourse.bass_utils import run_bass_kernel_spmd

F32 = mybir.dt.float32
BF16 = mybir.dt.bfloat16
EXP = mybir.ActivationFunctionType.Exp

# hardcoded problem shapes
T = 2048          # sequence length
C = 2048          # embedding dim
DH = 128          # head dim
NH = 16           # query heads
NKV = 4           # kv heads
N_CORES = 8
HPD = NH // N_CORES  # q-heads per device = 2
ROPE_BASE = 10000.0
SCALE = 1.0 / np.sqrt(DH)

NCT = C // 128    # 16 contraction tiles
NTC = T // 128    # 16 token chunks
SW = 256          # sub-quarter / attention window width
NSQ = T // SW     # 8 windows
WCT = HPD * DH + 2 * DH  # packed weight cols per c-tile = 512

BF = ml_dtypes.bfloat16


def _emit(nc):
    xT = nc.dram_tensor("xT", [C, T], BF16, kind="ExternalInput").ap()
    # packed per c-tile: [wq(2 heads) | wk | wv] = 512 cols each
    wqkv = nc.dram_tensor("wqkv", [128, NCT * WCT], BF16, kind="ExternalInput").ap()
    wp = nc.dram_tensor("wp", [128, HPD * C], BF16, kind="ExternalInput").ap()
    trig = nc.dram_tensor("trig", [128, 2 * T], BF16, kind="ExternalInput").ap()
    miscb = nc.dram_tensor("miscb", [128, 128 + 2 * SW], BF16,
                           kind="ExternalInput").ap()
    identf = nc.dram_tensor("identf", [128, 128], F32, kind="ExternalInput").ap()
    out = nc.dram_tensor("out", [T, C], BF16, kind="ExternalOutput").ap()

    with tile.TileContext(nc) as tc:
        with (
            tc.tile_pool(name="cst", bufs=1) as cst,
            tc.tile_pool(name="ps", bufs=1, space="PSUM") as ps,
            tc.tile_pool(name="xts", bufs=4) as xts,
            tc.tile_pool(name="rope", bufs=4) as rope,
            tc.tile_pool(name="pts", bufs=12) as pts,
            tc.tile_pool(name="osts", bufs=3) as osts,
            tc.tile_pool(name="rcs", bufs=2) as rcs,
        ):
            # persistent SBUF tensors
            trig_sb = cst.tile([128, 2 * T], BF16, tag="trig")
            cos_sb = trig_sb[:, 0:T]
            sin_sb = trig_sb[:, T:2 * T]
            misc_sb = cst.tile([128, 128 + 2 * SW], BF16, tag="miscb")
            ones_sb = misc_sb[:, 0:128]
            masks_sb = misc_sb[:, 128:128 + 2 * SW]
            identf_sb = cst.tile([128, 128], F32, tag="identf")
            scratch = cst.tile([128, 256], BF16, tag="scratch")
            wqkv_sb = cst.tile([128, NCT * WCT], BF16, tag="wqkv")
            wp_sb = cst.tile([128, HPD * C], BF16, tag="wp")

            qt_sb = [cst.tile([128, T], BF16, tag=f"qt{m}", name=f"qt{m}")
                     for m in range(HPD)]
            kt_sb = cst.tile([128, T], BF16, tag="kt")
            v_sb = cst.tile([128, NTC * DH], BF16, tag="v")
            yt_sb = [cst.tile([128, T], BF16, tag=f"yt{m}", name=f"yt{m}")
                     for m in range(HPD)]

            def wq_ap(ct, m):
                return wqkv_sb[:, ct * WCT + m * DH:ct * WCT + (m + 1) * DH]

            def wk_ap(ct):
                return wqkv_sb[:, ct * WCT + HPD * DH:ct * WCT + HPD * DH + DH]

            def wv_ap(ct):
                return wqkv_sb[:, ct * WCT + HPD * DH + DH:(ct + 1) * WCT]

            # ---- PE warmup: ramp the p-state and fill startup DMA stalls ----
            nc.vector.memset(scratch[:], 0)
            warm_state = {}

            def warm(n):
                if "ps" not in warm_state:
                    warm_state["ps"] = ps.tile([128, 512], F32, tag="sc",
                                               bufs=2, name="warm")
                for _ in range(n):
                    nc.tensor.matmul(warm_state["ps"][:, 0:256],
                                     scratch[:, 0:128], scratch[:],
                                     start=True, stop=True,
                                     skip_group_check=True)
            warm(14)

            # ---- DMA emission helpers ----
            xt_tiles = {}

            def emit_xt_dma(i, a, b):
                tsl = slice(i * SW, (i + 1) * SW)
                xt = xt_tiles[i]
                nc.sync.dma_start(
                    xt[:, a * SW:b * SW].rearrange("p (ct t) -> p ct t", t=SW),
                    xT[a * 128:b * 128, tsl].rearrange("(ct p) t -> p ct t", p=128))

            def load_xt(i, split=False):
                xt_tiles[i] = xts.tile([128, NCT * SW], BF16, tag="xt",
                                       name=f"xt{i}")
                if split:
                    emit_xt_dma(i, 0, 3)
                else:
                    emit_xt_dma(i, 0, 16)

            def emit_startup_dmas():
                # single sync-ring stream ordered by first use; HWDGE issues
                # one DMA per ~625ns so order is everything here
                load_xt(0, split=True)
                nc.sync.dma_start(wqkv_sb[:, 0:2 * WCT], wqkv[:, 0:2 * WCT])
                emit_xt_dma(0, 3, 8)
                nc.sync.dma_start(wqkv_sb[:, 2 * WCT:9 * WCT],
                                  wqkv[:, 2 * WCT:9 * WCT])
                emit_xt_dma(0, 8, 16)
                nc.sync.dma_start(trig_sb[:, 0:T], trig[:, 0:T])
                nc.sync.dma_start(wqkv_sb[:, 9 * WCT:16 * WCT],
                                  wqkv[:, 9 * WCT:16 * WCT])
                nc.sync.dma_start(trig_sb[:, T:2 * T], trig[:, T:2 * T])
                load_xt(1)
                nc.sync.dma_start(misc_sb[:], miscb[:])
                nc.sync.dma_start(identf_sb[:], identf[:])
                load_xt(2)
                nc.sync.dma_start(wp_sb[:], wp[:])

            # ---- projection units (PE filler) ----
            proj_psum = {}

            def proj_ct_unit(i, ct):
                def unit():
                    if ct == 0:
                        proj_psum[i] = (
                            ps.tile([128, 512], F32, tag="pqq", bufs=1,
                                    name=f"pqq{i}"),
                            ps.tile([128, 512], F32, tag="pkv", bufs=1,
                                    name=f"pkv{i}"),
                        )
                    pqq, pkv = proj_psum[i]
                    xt = xt_tiles[i]
                    xsl = slice(ct * SW, (ct + 1) * SW)
                    st = ct == 0
                    sp = ct == NCT - 1
                    for m in range(HPD):
                        nc.tensor.matmul(
                            pqq[:, m * SW:(m + 1) * SW], wq_ap(ct, m),
                            xt[:, xsl], start=st, stop=sp, skip_group_check=True)
                    nc.tensor.matmul(
                        pkv[:, 0:SW], wk_ap(ct),
                        xt[:, xsl], start=st, stop=sp, skip_group_check=True)
                    # v in [token, dh] layout: x chunk stationary, wv moving
                    for j in range(2):
                        nc.tensor.matmul(
                            pkv[:, SW + j * DH:SW + (j + 1) * DH],
                            xt[:, ct * SW + j * 128:ct * SW + (j + 1) * 128],
                            wv_ap(ct), start=st, stop=sp, skip_group_check=True)
                    if sp:
                        emit_rope_v(i)
                return unit

            def emit_rope_v(i):
                # rope: copy psum->sbuf bf16 (Pool), then 2x-mode DVE ops
                tsl = slice(i * SW, (i + 1) * SW)
                pqq, pkv = proj_psum[i]
                for idx, dst in [(0, qt_sb[0]), (1, qt_sb[1]), (2, kt_sb)]:
                    src = pkv if dst is kt_sb else pqq
                    psl = slice(0, SW) if idx != 1 else slice(SW, 2 * SW)
                    p_sb = rope.tile([128, SW], BF16, tag="p_sb")
                    nc.gpsimd.tensor_copy(p_sb[:], src[:, psl])
                    cr = rope.tile([128, SW], BF16, tag="cr")
                    nc.vector.tensor_mul(cr[:], p_sb[:], cos_sb[:, tsl])
                    ur = rope.tile([128, SW], BF16, tag="ur")
                    nc.vector.tensor_mul(ur[0:64, :], p_sb[64:128, :],
                                         sin_sb[0:64, tsl])
                    nc.vector.tensor_mul(ur[64:128, :], p_sb[0:64, :],
                                         sin_sb[64:128, tsl])
                    nc.vector.tensor_add(dst[:, tsl], cr[:], ur[:])
                # v: plain copies to [token, dh] sbuf
                for j in range(2):
                    tc0 = 2 * i + j
                    nc.gpsimd.tensor_copy(v_sb[:, tc0 * DH:(tc0 + 1) * DH],
                                          pkv[:, SW + j * DH:SW + (j + 1) * DH])

            # ---- output projection units (PE filler) ----
            OST_ENG = [nc.scalar.copy, nc.vector.tensor_copy,
                       nc.gpsimd.tensor_copy, nc.vector.tensor_copy]

            PS_BUFS = {"po": 2, "sc": 2, "ys": 2, "pqq": 1, "pkv": 1}

            def make_po_units(w, tag_cycle=("po",), split_dma=False):
                units = []
                uidx = [0]
                for j in range(2):
                    t0 = w * SW + j * 128
                    ost = osts.tile([128, C], BF16, tag="ost", name=f"ost{t0}")
                    for e in range(4):
                        def unit(t0=t0, ost=ost, e=e):
                            tg = tag_cycle[uidx[0] % len(tag_cycle)]
                            uidx[0] += 1
                            po = ps.tile([128, 512], F32, tag=tg,
                                         bufs=PS_BUFS[tg], name=f"po{t0}_{e}")
                            for k in range(HPD):
                                nc.tensor.matmul(
                                    po[:],
                                    yt_sb[k][:, t0:t0 + 128],
                                    wp_sb[:, k * C + e * 512:k * C + (e + 1) * 512],
                                    start=(k == 0), stop=(k == HPD - 1))
                            OST_ENG[e](ost[:, e * 512:(e + 1) * 512], po[:])
                        units.append(unit)

                    def dma_unit(t0=t0, ost=ost):
                        if split_dma:
                            for q in range(2):
                                nc.scalar.dma_start(
                                    out[t0:t0 + 128, q * 1024:(q + 1) * 1024],
                                    ost[:, q * 1024:(q + 1) * 1024])
                        else:
                            nc.scalar.dma_start(out[t0:t0 + 128, :], ost[:])
                    units.append(dma_unit)
                return units

            # ---- attention units ----
            def make_attn_units(w, sc_tags=("sc",)):
                """Window w, both heads: scores/exp/mask, y/s, rc chain."""
                units = []
                npair = w + 1
                tw0 = w * SW
                wsl = slice(tw0, tw0 + SW)
                scidx = [0]

                def sc_tile(name):
                    tg = sc_tags[scidx[0] % len(sc_tags)]
                    scidx[0] += 1
                    return ps.tile([128, 512], F32, tag=tg, bufs=PS_BUFS[tg],
                                   name=name)

                def y_pair(h, cp, state):
                    nch = 2 * npair
                    ys = state["ys"]
                    pt = state["pts"][cp]
                    for k2 in range(2):
                        cc = 2 * cp + k2
                        st = cc == 0
                        sp = cc == nch - 1
                        psl = slice(k2 * SW, (k2 + 1) * SW)
                        nc.tensor.matmul(
                            ys[:, 0:SW], v_sb[:, cc * DH:(cc + 1) * DH],
                            pt[:, psl], start=st, stop=sp,
                            skip_group_check=True)
                        for h2 in range(2):
                            nc.tensor.matmul(
                                ys[:, SW + h2:SW + h2 + 1],
                                pt[:, k2 * SW + h2 * 128:k2 * SW + (h2 + 1) * 128],
                                ones_sb[:, 0:1], start=st, stop=sp,
                                skip_group_check=True)

                def sc_unit(h, cp, state):
                    if cp == 0:
                        state["ys"] = ps.tile([128, 512], F32, tag="ys",
                                              bufs=2, name=f"ys{w}_{h}")
                    sc_ps = sc_tile(f"sc{w}_{h}_{cp}")
                    for k2 in range(2):
                        cc = 2 * cp + k2
                        nc.tensor.matmul(
                            sc_ps[:, k2 * SW:(k2 + 1) * SW],
                            kt_sb[:, cc * 128:(cc + 1) * 128],
                            qt_sb[h][:, wsl], start=True, stop=True)
                    pt = pts.tile([128, 512], BF16, tag="pt",
                                  name=f"pt{w}_{h}_{cp}")
                    nc.scalar.activation(pt[:], sc_ps[:], EXP,
                                         scale=float(SCALE))
                    if cp == npair - 1:
                        # diagonal pair: zero strictly-above-diagonal
                        nc.vector.tensor_mul(pt[:], pt[:], masks_sb[:])
                    state["pts"][cp] = pt
                    if cp >= 1:
                        y_pair(h, cp - 1, state)

                for h in range(HPD):
                    state = {"pts": {}, "ys": None}

                    for cp in range(npair):
                        units.append(lambda h=h, cp=cp, state=state:
                                     sc_unit(h, cp, state))

                    def y_last(h=h, state=state):
                        y_pair(h, npair - 1, state)
                        s_sb = rcs.tile([128, 2], F32, tag="s_sb",
                                        name=f"ssb{w}_{h}")
                        nc.vector.tensor_copy(s_sb[:], state["ys"][:, SW:SW + 2])
                        state["s_sb"] = s_sb
                    units.append(y_last)

                    def rc1(h=h, state=state):
                        rc_ps = sc_tile(f"rcps{w}_{h}")
                        for h2 in range(2):
                            nc.tensor.transpose(
                                rc_ps[0:1, h2 * 128:(h2 + 1) * 128],
                                state["s_sb"][:, h2:h2 + 1], identf_sb[:])
                        state["rc_ps"] = rc_ps
                        rcT_sb = rcs.tile([1, 256], BF16, tag="rcT",
                                          name=f"rcT{w}_{h}")
                        with nc.allow_low_precision("softmax denominators"):
                            nc.vector.reciprocal(rcT_sb[:], rc_ps[0:1, 0:256])
                        state["rcT_sb"] = rcT_sb
                    units.append(rc1)

                    def rc2(h=h, state=state):
                        rc_ps = state["rc_ps"]
                        for h2 in range(2):
                            nc.tensor.matmul(
                                rc_ps[:, 256 + h2 * 128:256 + (h2 + 1) * 128],
                                ones_sb[0:1, :],
                                state["rcT_sb"][:, h2 * 128:(h2 + 1) * 128],
                                start=True, stop=True, skip_group_check=True)
                        nc.vector.tensor_mul(yt_sb[h][:, wsl],
                                             state["ys"][:, 0:SW],
                                             rc_ps[:, 256:512])
                    units.append(rc2)
                return units

            def weave(a_units, f_units):
                # distribute fillers evenly across the attention stream
                na, nf = len(a_units), len(f_units)
                if na == 0:
                    for f in f_units:
                        f()
                    return
                taken = 0
                for idx, a in enumerate(a_units):
                    a()
                    want = min(nf, ((idx + 1) * nf) // na)
                    while taken < want:
                        f_units[taken]()
                        taken += 1

            # ---------------- main schedule ----------------
            emit_startup_dmas()
            for i in range(NSQ):
                if i + 3 < NSQ:
                    load_xt(i + 3)
                f_units = [proj_ct_unit(i, ct) for ct in range(NCT)]
                if i == 0:
                    # interpose no-dep warm matmuls where the weight/x DMA
                    # stream gates proj(0); the PE is in-order so filler must
                    # precede each stall point
                    f2 = []
                    for ct, u in enumerate(f_units):
                        if ct == 2:
                            f2.append(lambda: warm(4))
                        if ct == 9:
                            f2.append(lambda: warm(10))
                        f2.append(u)
                    f2.append(lambda: warm(6))
                    f_units = f2
                if i >= 2:
                    f_units += make_po_units(i - 2)
                a_units = make_attn_units(i - 1) if i >= 1 else []
                weave(a_units, f_units)
            # tail: window 7 + outproj windows 6, 7; pqq/pkv banks are dead
            # here so attention scores and the last outproj rotate over them
            weave(make_attn_units(NSQ - 1, sc_tags=("sc", "pqq", "pkv")),
                  make_po_units(NSQ - 2))
            for u in make_po_units(NSQ - 1, tag_cycle=("po", "sc", "ys", "pqq"),
                                   split_dma=True):
                u()

    nc.compile()
    return nc


_CACHE = {}


def _get_module():
    if "nc" not in _CACHE:
        nc = bacc.Bacc("TRN2", target_bir_lowering=False, debug=False)
        _CACHE["nc"] = _emit(nc)
    return _CACHE["nc"]


def _host_constants():
    if "consts" in _CACHE:
        return _CACHE["consts"]
    inv_freq = 1.0 / (ROPE_BASE ** (np.arange(0, DH, 2, dtype=np.float64) / DH))
    ang = np.outer(np.arange(T, dtype=np.float64), inv_freq)      # (T, 64)
    emb = np.concatenate([ang, ang], axis=-1)                     # (T, 128)
    cos = np.cos(emb).astype(np.float32)                          # (T, 128)
    sin = np.sin(emb).astype(np.float32)
    cosT = np.ascontiguousarray(cos.T)                            # (128, T)
    sign = np.where(np.arange(DH) < DH // 2, -1.0, 1.0).astype(np.float32)
    sinR = np.ascontiguousarray(sin.T * sign[:, None])
    trig = np.concatenate([cosT, sinR], axis=1).astype(BF)        # (128, 2T)
    ones = np.ones((128, 128), dtype=np.float32)
    identf = np.eye(128, dtype=np.float32)
    # causal masks for the diagonal chunk pair of each 256-wide window
    j = np.arange(SW)[None, :]
    p = np.arange(128)[:, None]
    m0 = (j >= p).astype(np.float32)
    m1 = (j >= p + 128).astype(np.float32)
    miscb = np.concatenate([ones, m0, m1], axis=1).astype(BF)     # (128, 640)
    _CACHE["consts"] = (trig, miscb, identf)
    return _CACHE["consts"]


def kernel(x, wq, wk, wv, wproj):
    x = np.asarray(x, dtype=np.float32)
    wq = np.asarray(wq, dtype=np.float32)
    wk = np.asarray(wk, dtype=np.float32)
    wv = np.asarray(wv, dtype=np.float32)
    wproj = np.asarray(wproj, dtype=np.float32)

    nc = _get_module()
    trig, miscb, identf = _host_constants()
    xT = np.ascontiguousarray(x[0].T).astype(BF)                  # (C, T)

    in_maps = []
    for d in range(N_CORES):
        h0 = HPD * d
        g = d // 2
        # packed weights per c-tile: [wq(2 heads) | wk | wv]
        wq_d = wq[:, h0 * DH:(h0 + HPD) * DH].reshape(NCT, 128, HPD * DH)
        wk_d = wk[:, g * DH:(g + 1) * DH].reshape(NCT, 128, DH)
        wv_d = wv[:, g * DH:(g + 1) * DH].reshape(NCT, 128, DH)
        wqkv_d = np.concatenate([wq_d, wk_d, wv_d], axis=2)       # (NCT,128,512)
        wqkv_l = np.ascontiguousarray(
            wqkv_d.transpose(1, 0, 2).reshape(128, -1)).astype(BF)
        # wproj rows for our heads -> [128, HPD*C] (head-major free dim)
        wp_d = wproj[h0 * DH:(h0 + HPD) * DH, :]                  # (HPD*DH, C)
        wp_l = np.ascontiguousarray(
            wp_d.reshape(HPD, 128, C).transpose(1, 0, 2).reshape(128, -1)
        ).astype(BF)
        in_maps.append({
            "xT": xT, "wqkv": wqkv_l, "wp": wp_l,
            "trig": trig, "miscb": miscb, "identf": identf,
        })

    res = run_bass_kernel_spmd(nc, in_maps, core_ids=list(range(N_CORES)))
    acc = res.results[0]["out"].astype(np.float32)
    for d in range(1, N_CORES):
        acc = acc + res.results[d]["out"].astype(np.float32)
    return acc.reshape(1, T, C)


# revision 26
# speedup vs baseline: 1.1829x; 1.0049x over previous
"""Trainium2 Bass kernel for CausalSelfAttentionModern (GQA + RoPE + causal SDPA).

Sharding: tensor-parallel over heads across 8 NeuronCores.
Device d owns q-heads {2d, 2d+1} and kv-head d//2.
Each device computes its heads' attention plus its slice of the output
projection (row-parallel); the host sums the 8 partial outputs.

v3: bf16 data path (DMA + matmul inputs; fp32 PSUM accumulation),
softmax denominator via tiny stationary-pt matmuls instead of a full
ones-matmul, V projected directly in [token, dh] layout, causal mask via
constant-mask multiply, packed weight/constant tensors to minimize DMA
trigger count (HWDGE is 625ns/DMA serialized), and a unified PE weave:
projection of sub-quarter i and output projection of window i-2 fill the
latency bubbles of attention window i-1.
"""

import numpy as np
import ml_dtypes
import concourse.bacc as bacc
import concourse.tile as tile
import concourse.mybir as mybir
from concourse.bass_utils import run_bass_kernel_spmd

F32 = mybir.dt.float32
BF16 = mybir.dt.bfloat16
EXP = mybir.ActivationFunctionType.Exp

# hardcoded problem shapes
T = 2048          # sequence length
C = 2048          # embedding dim
DH = 128          # head dim
NH = 16           # query heads
NKV = 4           # kv heads
N_CORES = 8
HPD = NH // N_CORES  # q-heads per device = 2
ROPE_BASE = 10000.0
SCALE = 1.0 / np.sqrt(DH)

NCT = C // 128    # 16 contraction tiles
NTC = T // 128    # 16 token chunks
SW = 256          # sub-quarter / attention window width
NSQ = T // SW     # 8 windows
WCT = HPD * DH + 2 * DH  # packed weight cols per c-tile = 512

BF = ml_dtypes.bfloat16


def _emit(nc):
    xT = nc.dram_tensor("xT", [C, T], BF16, kind="ExternalInput").ap()
    # packed per c-tile: [wq(2 heads) | wk | wv] = 512 cols each
    wqkv = nc.dram_tensor("wqkv", [128, NCT * WCT], BF16, kind="ExternalInput").ap()
    wp = nc.dram_tensor("wp", [128, HPD * C], BF16, kind="ExternalInput").ap()
    trig = nc.dram_tensor("trig", [128, 2 * T], BF16, kind="ExternalInput").ap()
    miscb = nc.dram_tensor("miscb", [128, 128 + 2 * SW], BF16,
                           kind="ExternalInput").ap()
    identf = nc.dram_tensor("identf", [128, 128], F32, kind="ExternalInput").ap()
    out = nc.dram_tensor("out", [T, C], BF16, kind="ExternalOutput").ap()

    with tile.TileContext(nc) as tc:
        with (
            tc.tile_pool(name="cst", bufs=1) as cst,
            tc.tile_pool(name="ps", bufs=1, space="PSUM") as ps,
            tc.tile_pool(name="xts", bufs=4) as xts,
            tc.tile_pool(name="rope", bufs=4) as rope,
            tc.tile_pool(name="pts", bufs=12) as pts,
            tc.tile_pool(name="osts", bufs=3) as osts,
            tc.tile_pool(name="rcs", bufs=2) as rcs,
        ):
            # persistent SBUF tensors
            trig_sb = cst.tile([128, 2 * T], BF16, tag="trig")
            cos_sb = trig_sb[:, 0:T]
            sin_sb = trig_sb[:, T:2 * T]
            misc_sb = cst.tile([128, 128 + 2 * SW], BF16, tag="miscb")
            ones_sb = misc_sb[:, 0:128]
            masks_sb = misc_sb[:, 128:128 + 2 * SW]
            identf_sb = cst.tile([128, 128], F32, tag="identf")
            scratch = cst.tile([128, 256], BF16, tag="scratch")
            wqkv_sb = cst.tile([128, NCT * WCT], BF16, tag="wqkv")
            wp_sb = cst.tile([128, HPD * C], BF16, tag="wp")

            qt_sb = [cst.tile([128, T], BF16, tag=f"qt{m}", name=f"qt{m}")
                     for m in range(HPD)]
            kt_sb = cst.tile([128, T], BF16, tag="kt")
            v_sb = cst.tile([128, NTC * DH], BF16, tag="v")
            yt_sb = [cst.tile([128, T], BF16, tag=f"yt{m}", name=f"yt{m}")
                     for m in range(HPD)]

            def wq_ap(ct, m):
                return wqkv_sb[:, ct * WCT + m * DH:ct * WCT + (m + 1) * DH]

            def wk_ap(ct):
                return wqkv_sb[:, ct * WCT + HPD * DH:ct * WCT + HPD * DH + DH]

            def wv_ap(ct):
                return wqkv_sb[:, ct * WCT + HPD * DH + DH:(ct + 1) * WCT]

            # ---- PE warmup: ramp the p-state and fill startup DMA stalls ----
            nc.vector.memset(scratch[:], 0)
            warm_state = {}

            def warm(n):
                if "ps" not in warm_state:
                    warm_state["ps"] = ps.tile([128, 512], F32, tag="sc",
                                               bufs=2, name="warm")
                for _ in range(n):
                    nc.tensor.matmul(warm_state["ps"][:, 0:256],
                                     scratch[:, 0:128], scratch[:],
                                     start=True, stop=True,
                                     skip_group_check=True)
            warm(14)

            # ---- DMA emission helpers ----
            xt_tiles = {}

            def emit_xt_dma(i, a, b):
                tsl = slice(i * SW, (i + 1) * SW)
                xt = xt_tiles[i]
                nc.sync.dma_start(
                    xt[:, a * SW:b * SW].rearrange("p (ct t) -> p ct t", t=SW),
                    xT[a * 128:b * 128, tsl].rearrange("(ct p) t -> p ct t", p=128))

            def load_xt(i, split=False):
                xt_tiles[i] = xts.tile([128, NCT * SW], BF16, tag="xt",
                                       name=f"xt{i}")
                if split:
                    emit_xt_dma(i, 0, 3)
                else:
                    emit_xt_dma(i, 0, 16)

            def emit_startup_dmas():
                # single sync-ring stream ordered by first use; HWDGE issues
                # one DMA per ~625ns so order is everything here
                load_xt(0, split=True)
                nc.sync.dma_start(wqkv_sb[:, 0:2 * WCT], wqkv[:, 0:2 * WCT])
                emit_xt_dma(0, 3, 8)
                nc.sync.dma_start(wqkv_sb[:, 2 * WCT:9 * WCT],
                                  wqkv[:, 2 * WCT:9 * WCT])
                emit_xt_dma(0, 8, 16)
                nc.sync.dma_start(wqkv_sb[:, 9 * WCT:16 * WCT],
                                  wqkv[:, 9 * WCT:16 * WCT])
                nc.sync.dma_start(trig_sb[:, 0:T], trig[:, 0:T])
                nc.sync.dma_start(trig_sb[:, T:2 * T], trig[:, T:2 * T])
                load_xt(1)
                nc.sync.dma_start(misc_sb[:], miscb[:])
                nc.sync.dma_start(identf_sb[:], identf[:])
                load_xt(2)
                nc.sync.dma_start(wp_sb[:], wp[:])

            # ---- projection units (PE filler) ----
            proj_psum = {}

            def proj_ct_unit(i, ct):
                def unit():
                    if ct == 0:
                        proj_psum[i] = (
                            ps.tile([128, 512], F32, tag="pqq", bufs=1,
                                    name=f"pqq{i}"),
                            ps.tile([128, 512], F32, tag="pkv", bufs=1,
                                    name=f"pkv{i}"),
                        )
                    pqq, pkv = proj_psum[i]
                    xt = xt_tiles[i]
                    xsl = slice(ct * SW, (ct + 1) * SW)
                    st = ct == 0
                    sp = ct == NCT - 1
                    for m in range(HPD):
                        nc.tensor.matmul(
                            pqq[:, m * SW:(m + 1) * SW], wq_ap(ct, m),
                            xt[:, xsl], start=st, stop=sp, skip_group_check=True)
                    nc.tensor.matmul(
                        pkv[:, 0:SW], wk_ap(ct),
                        xt[:, xsl], start=st, stop=sp, skip_group_check=True)
                    # v in [token, dh] layout: x chunk stationary, wv moving
                    for j in range(2):
                        nc.tensor.matmul(
                            pkv[:, SW + j * DH:SW + (j + 1) * DH],
                            xt[:, ct * SW + j * 128:ct * SW + (j + 1) * 128],
                            wv_ap(ct), start=st, stop=sp, skip_group_check=True)
                    if sp:
                        emit_rope_v(i)
                return unit

            def emit_rope_v(i):
                # rope: copy psum->sbuf bf16 (Pool), then 2x-mode DVE ops
                tsl = slice(i * SW, (i + 1) * SW)
                pqq, pkv = proj_psum[i]
                for idx, dst in [(0, qt_sb[0]), (1, qt_sb[1]), (2, kt_sb)]:
                    src = pkv if dst is kt_sb else pqq
                    psl = slice(0, SW) if idx != 1 else slice(SW, 2 * SW)
                    p_sb = rope.tile([128, SW], BF16, tag="p_sb")
                    nc.gpsimd.tensor_copy(p_sb[:], src[:, psl])
                    cr = rope.tile([128, SW], BF16, tag="cr")
                    nc.vector.tensor_mul(cr[:], p_sb[:], cos_sb[:, tsl])
                    ur = rope.tile([128, SW], BF16, tag="ur")
                    nc.vector.tensor_mul(ur[0:64, :], p_sb[64:128, :],
                                         sin_sb[0:64, tsl])
                    nc.vector.tensor_mul(ur[64:128, :], p_sb[0:64, :],
                                         sin_sb[64:128, tsl])
                    nc.vector.tensor_add(dst[:, tsl], cr[:], ur[:])
                # v: plain copies to [token, dh] sbuf
                for j in range(2):
                    tc0 = 2 * i + j
                    nc.gpsimd.tensor_copy(v_sb[:, tc0 * DH:(tc0 + 1) * DH],
                                          pkv[:, SW + j * DH:SW + (j + 1) * DH])

            # ---- output projection units (PE filler) ----
            OST_ENG = [nc.scalar.copy, nc.vector.tensor_copy,
                       nc.gpsimd.tensor_copy, nc.vector.tensor_copy]
            OST_ENG_NOACT = [nc.vector.tensor_copy, nc.gpsimd.tensor_copy,
                             nc.vector.tensor_copy, nc.gpsimd.tensor_copy]

            PS_BUFS = {"po": 2, "sc": 2, "ys": 2, "pqq": 1, "pkv": 1}

            def make_po_units(w, tag_cycle=("po",), split_dma=False,
                              no_act=False):
                units = []
                uidx = [0]
                for j in range(2):
                    t0 = w * SW + j * 128
                    ost = osts.tile([128, C], BF16, tag="ost", name=f"ost{t0}")
                    for e in range(4):
                        def unit(t0=t0, ost=ost, e=e):
                            tg = tag_cycle[uidx[0] % len(tag_cycle)]
                            uidx[0] += 1
                            po = ps.tile([128, 512], F32, tag=tg,
                                         bufs=PS_BUFS[tg], name=f"po{t0}_{e}")
                            for k in range(HPD):
                                nc.tensor.matmul(
                                    po[:],
                                    yt_sb[k][:, t0:t0 + 128],
                                    wp_sb[:, k * C + e * 512:k * C + (e + 1) * 512],
                                    start=(k == 0), stop=(k == HPD - 1))
                            eng = OST_ENG_NOACT if no_act else OST_ENG
                            eng[e](ost[:, e * 512:(e + 1) * 512], po[:])
                            if split_dma:
                                nc.scalar.dma_start(
                                    out[t0:t0 + 128, e * 512:(e + 1) * 512],
                                    ost[:, e * 512:(e + 1) * 512])
                        units.append(unit)

                    if not split_dma:
                        def dma_unit(t0=t0, ost=ost):
                            nc.scalar.dma_start(out[t0:t0 + 128, :], ost[:])
                        units.append(dma_unit)
                return units

            # ---- attention units ----
            def make_attn_units(w, sc_tags=("sc",)):
                """Window w, both heads: scores/exp/mask, y/s, rc chain."""
                units = []
                npair = w + 1
                tw0 = w * SW
                wsl = slice(tw0, tw0 + SW)
                scidx = [0]

                def sc_tile(name):
                    tg = sc_tags[scidx[0] % len(sc_tags)]
                    scidx[0] += 1
                    return ps.tile([128, 512], F32, tag=tg, bufs=PS_BUFS[tg],
                                   name=name)

                def y_pair(h, cp, state):
                    nch = 2 * npair
                    ys = state["ys"]
                    pt = state["pts"][cp]
                    for k2 in range(2):
                        cc = 2 * cp + k2
                        st = cc == 0
                        sp = cc == nch - 1
                        psl = slice(k2 * SW, (k2 + 1) * SW)
                        nc.tensor.matmul(
                            ys[:, 0:SW], v_sb[:, cc * DH:(cc + 1) * DH],
                            pt[:, psl], start=st, stop=sp,
                            skip_group_check=True)
                        for h2 in range(2):
                            nc.tensor.matmul(
                                ys[:, SW + h2:SW + h2 + 1],
                                pt[:, k2 * SW + h2 * 128:k2 * SW + (h2 + 1) * 128],
                                ones_sb[:, 0:1], start=st, stop=sp,
                                skip_group_check=True)

                def sc_unit(h, cp, state):
                    if cp == 0:
                        state["ys"] = ps.tile([128, 512], F32, tag="ys",
                                              bufs=2, name=f"ys{w}_{h}")
                    sc_ps = sc_tile(f"sc{w}_{h}_{cp}")
                    for k2 in range(2):
                        cc = 2 * cp + k2
                        nc.tensor.matmul(
                            sc_ps[:, k2 * SW:(k2 + 1) * SW],
                            kt_sb[:, cc * 128:(cc + 1) * 128],
                            qt_sb[h][:, wsl], start=True, stop=True)
                    pt = pts.tile([128, 512], BF16, tag="pt",
                                  name=f"pt{w}_{h}_{cp}")
                    nc.scalar.activation(pt[:], sc_ps[:], EXP,
                                         scale=float(SCALE))
                    if cp == npair - 1:
                        # diagonal pair: zero strictly-above-diagonal
                        nc.vector.tensor_mul(pt[:], pt[:], masks_sb[:])
                    state["pts"][cp] = pt
                    if cp >= 1:
                        y_pair(h, cp - 1, state)

                def head_units(h):
                    state = {"pts": {}, "ys": None}
                    hu = [lambda h=h, cp=cp, state=state: sc_unit(h, cp, state)
                          for cp in range(npair)]

                    def y_last(h=h, state=state):
                        y_pair(h, npair - 1, state)
                        s_sb = rcs.tile([128, 2], F32, tag="s_sb",
                                        name=f"ssb{w}_{h}")
                        nc.vector.tensor_copy(s_sb[:], state["ys"][:, SW:SW + 2])
                        state["s_sb"] = s_sb

                    def rc1(h=h, state=state):
                        rc_ps = sc_tile(f"rcps{w}_{h}")
                        for h2 in range(2):
                            nc.tensor.transpose(
                                rc_ps[0:1, h2 * 128:(h2 + 1) * 128],
                                state["s_sb"][:, h2:h2 + 1], identf_sb[:])
                        state["rc_ps"] = rc_ps
                        rcT_sb = rcs.tile([1, 256], BF16, tag="rcT",
                                          name=f"rcT{w}_{h}")
                        with nc.allow_low_precision("softmax denominators"):
                            nc.vector.reciprocal(rcT_sb[:], rc_ps[0:1, 0:256])
                        state["rcT_sb"] = rcT_sb

                    def rc2(h=h, state=state):
                        rc_ps = state["rc_ps"]
                        for h2 in range(2):
                            nc.tensor.matmul(
                                rc_ps[:, 256 + h2 * 128:256 + (h2 + 1) * 128],
                                ones_sb[0:1, :],
                                state["rcT_sb"][:, h2 * 128:(h2 + 1) * 128],
                                start=True, stop=True, skip_group_check=True)
                        nc.vector.tensor_mul(yt_sb[h][:, wsl],
                                             state["ys"][:, 0:SW],
                                             rc_ps[:, 256:512])
                    return hu, [y_last, rc1, rc2]

                # h0's rc chain interleaves into h1's score stream so its
                # DVE latencies hide behind h1's PE work
                h0_sc, h0_rc = head_units(0)
                h1_sc, h1_rc = head_units(1)
                units += h0_sc
                units.append(h0_rc[0])          # y_last(h0)
                merged = 0
                for k, u in enumerate(h1_sc):
                    units.append(u)
                    if merged < 2 and k < len(h1_sc) - 1:
                        units.append(h0_rc[1 + merged])
                        merged += 1
                while merged < 2:
                    units.append(h0_rc[1 + merged])
                    merged += 1
                units += h1_rc
                return units

            def weave(a_units, f_units):
                # distribute fillers evenly across the attention stream
                na, nf = len(a_units), len(f_units)
                if na == 0:
                    for f in f_units:
                        f()
                    return
                taken = 0
                for idx, a in enumerate(a_units):
                    a()
                    want = min(nf, ((idx + 1) * nf) // na)
                    while taken < want:
                        f_units[taken]()
                        taken += 1

            # ---------------- main schedule ----------------
            emit_startup_dmas()
            for i in range(NSQ):
                if i + 3 < NSQ:
                    load_xt(i + 3)
                f_units = [proj_ct_unit(i, ct) for ct in range(NCT)]
                if i == 0:
                    # interpose no-dep warm matmuls where the weight/x DMA
                    # stream gates proj(0); the PE is in-order so filler must
                    # precede each stall point
                    f2 = []
                    for ct, u in enumerate(f_units):
                        if ct == 2:
                            f2.append(lambda: warm(4))
                        if ct == 9:
                            f2.append(lambda: warm(14))
                        f2.append(u)
                    f2.append(lambda: warm(12))
                    f_units = f2
                if i >= 2:
                    f_units += make_po_units(i - 2)
                a_units = make_attn_units(i - 1) if i >= 1 else []
                weave(a_units, f_units)
            # tail: window 7 + outproj windows 6, 7; pqq/pkv banks are dead
            # here so attention scores and the last outproj rotate over them
            weave(make_attn_units(NSQ - 1, sc_tags=("sc", "pqq", "pkv")),
                  make_po_units(NSQ - 2, no_act=True))
            for u in make_po_units(NSQ - 1, tag_cycle=("po", "sc", "ys", "pqq"),
                                   split_dma=True):
                u()

    nc.compile()
    return nc


_CACHE = {}


def _get_module():
    if "nc" not in _CACHE:
        nc = bacc.Bacc("TRN2", target_bir_lowering=False, debug=False)
        _CACHE["nc"] = _emit(nc)
    return _CACHE["nc"]


def _host_constants():
    if "consts" in _CACHE:
        return _CACHE["consts"]
    inv_freq = 1.0 / (ROPE_BASE ** (np.arange(0, DH, 2, dtype=np.float64) / DH))
    ang = np.outer(np.arange(T, dtype=np.float64), inv_freq)      # (T, 64)
    emb = np.concatenate([ang, ang], axis=-1)                     # (T, 128)
    cos = np.cos(emb).astype(np.float32)                          # (T, 128)
    sin = np.sin(emb).astype(np.float32)
    cosT = np.ascontiguousarray(cos.T)                            # (128, T)
    sign = np.where(np.arange(DH) < DH // 2, -1.0, 1.0).astype(np.float32)
    sinR = np.ascontiguousarray(sin.T * sign[:, None])
    trig = np.concatenate([cosT, sinR], axis=1).astype(BF)        # (128, 2T)
    ones = np.ones((128, 128), dtype=np.float32)
    identf = np.eye(128, dtype=np.float32)
    # causal masks for the diagonal chunk pair of each 256-wide window
    j = np.arange(SW)[None, :]
    p = np.arange(128)[:, None]
    m0 = (j >= p).astype(np.float32)
    m1 = (j >= p + 128).astype(np.float32)
    miscb = np.concatenate([ones, m0, m1], axis=1).astype(BF)     # (128, 640)
    _CACHE["consts"] = (trig, miscb, identf)
    return _CACHE["consts"]


def kernel(x, wq, wk, wv, wproj):
    x = np.asarray(x, dtype=np.float32)
    wq = np.asarray(wq, dtype=np.float32)
    wk = np.asarray(wk, dtype=np.float32)
    wv = np.asarray(wv, dtype=np.float32)
    wproj = np.asarray(wproj, dtype=np.float32)

    nc = _get_module()
    trig, miscb, identf = _host_constants()
    xT = np.ascontiguousarray(x[0].T).astype(BF)                  # (C, T)

    in_maps = []
    for d in range(N_CORES):
        h0 = HPD * d
        g = d // 2
        # packed weights per c-tile: [wq(2 heads) | wk | wv]
        wq_d = wq[:, h0 * DH:(h0 + HPD) * DH].reshape(NCT, 128, HPD * DH)
        wk_d = wk[:, g * DH:(g + 1) * DH].reshape(NCT, 128, DH)
        wv_d = wv[:, g * DH:(g + 1) * DH].reshape(NCT, 128, DH)
        wqkv_d = np.concatenate([wq_d, wk_d, wv_d], axis=2)       # (NCT,128,512)
        wqkv_l = np.ascontiguousarray(
            wqkv_d.transpose(1, 0, 2).reshape(128, -1)).astype(BF)
        # wproj rows for our heads -> [128, HPD*C] (head-major free dim)
        wp_d = wproj[h0 * DH:(h0 + HPD) * DH, :]                  # (HPD*DH, C)
        wp_l = np.ascontiguousarray(
            wp_d.reshape(HPD, 128, C).transpose(1, 0, 2).reshape(128, -1)
        ).astype(BF)
        in_maps.append({
            "xT": xT, "wqkv": wqkv_l, "wp": wp_l,
            "trig": trig, "miscb": miscb, "identf": identf,
        })

    res = run_bass_kernel_spmd(nc, in_maps, core_ids=list(range(N_CORES)))
    acc = res.results[0]["out"].astype(np.float32)
    for d in range(1, N_CORES):
        acc = acc + res.results[d]["out"].astype(np.float32)
    return acc.reshape(1, T, C)
